# revision 2
# baseline (speedup 1.0000x reference)
"""Trainium2 Bass kernel for nn_DifferentialGQA (8-core SPMD), v3.

Fully-fused single-phase pipeline (tensor-parallel, no mid-kernel AllReduce):
  - lambda is computed EXACTLY on the host (it is a linear+rope functional of
    x and the weights), so the AllReduce and the phase-A/phase-C serialization
    of v2 disappear entirely.
  - Per 128-row block lg: QKV matmuls (PE) -> rope (DVE stage + GPSIMD) ->
    q/k transposes (PE) -> attention fronts for qb=lg (score matmuls +
    diag-mask matmul; ACT exp with free row-sum accumulation) -> backs
    trailing one lg (fused diff stt on DVE, PE transposes + PV) -> per-qb RMS
    + output transpose trailing three lgs. ACT's exp stream overlaps the QKV
    GEMMs; PE stays continuously busy (p-state friendly).
  - Engine budget: PE ~117us (the roofline), ACT ~97us (exp+accum), DVE ~85us
    (copies + fused diff), Pool ~65us (ropes + small elementwise).
  - x^T is host-swizzled so each lg-major stream chunk is 512B-contiguous
    (full DMA rate); PE starts after ~6us.
  - Output reshard uses TWO AllToAlls over an interleaved qb->core map
    (core c owns L rows of q-blocks c and c+8): the first fires mid-kernel
    (fully hidden), the second at the end overlaps the first Wo half.
  - RMS rsqrt via ln+exp (both live in the natural_log_exp activation table:
    zero table swaps).
"""
import sys

sys.path.insert(0, "/opt/trn_rl_repo")

import numpy as np
import ml_dtypes

import concourse.bass as bass
import concourse.mybir as mybir
import concourse.tile as tile
from concourse import bacc
from concourse.bass_utils import run_bass_kernel_spmd
from concourse.hw_specs import get_activation_tables
from concourse.masks import make_identity

dt = mybir.dt
AF = mybir.ActivationFunctionType
OP = mybir.AluOpType

N_CORES = 8
L = 2048
HID = 2048
H = 32
HKV = 8
D = 64
CAP = 50.0
LAMBDA_INIT = 0.8 - 0.6 * float(np.exp(-0.3 * 4))
P = 128
LROWS = L // N_CORES          # 256 output rows per core
NQB = L // P                  # 16 query blocks
KT = HID // P                 # 16 contraction tiles
NH = H // N_CORES             # 4 q heads per core
SCALE = 1.0 / float(np.sqrt(D))
SCHUNK = 1024                 # exp chunk (2 PSUM banks of f32)


def _build(mock_collectives: bool = False, debug: bool = False):
    nc = bacc.Bacc("TRN2", target_bir_lowering=False, debug=False,
                   num_devices=(1 if mock_collectives else N_CORES))
    f32, bf16 = dt.float32, dt.bfloat16

    xtp = nc.dram_tensor("xtp", [NQB * 8 * P, 256], bf16,
                         kind="ExternalInput").ap()
    wqkv = nc.dram_tensor("wqkv", [HID, 384], bf16, kind="ExternalInput").ap()
    ropet = nc.dram_tensor("ropet", [L, 64], f32, kind="ExternalInput").ap()
    lam_in = nc.dram_tensor("lamneg", [1, 1], f32, kind="ExternalInput").ap()
    wo = nc.dram_tensor("wo", [H * D // 2, HID], bf16, kind="ExternalInput").ap()
    out_d = nc.dram_tensor("out", [LROWS, HID], bf16, kind="ExternalOutput").ap()
    dbg = {}
    if debug:
        for nm, shp, dty in [
            ("d_q0", [P, 256], f32), ("d_k0", [P, D], f32),
            ("d_qT0", [P, L], f32), ("d_kT", [P, L], f32),
            ("d_vm", [P, D], f32), ("d_lam", [P, 1], f32),
            ("d_r1", [P, 32], f32), ("d_r2", [P, 32], f32),
            ("d_e0", [P, L], f32), ("d_diff3", [P, L], f32),
            ("d_out1", [P, NQB, P], f32), ("d_ssq", [P, 32], f32),
            ("d_scl", [P, 32], f32), ("d_onT", [P, L], f32),
        ]:
            dbg[nm] = nc.dram_tensor(nm, shp, dty, kind="ExternalOutput").ap()

    with tile.TileContext(nc) as tc:
        with (
            tc.tile_pool(name="persist", bufs=1) as pp,
            tc.tile_pool(name="dram", bufs=1, space="DRAM") as dram,
        ):
            a2_inA = dram.tile([N_CORES * P, P], bf16, tag="a2_inA")
            a2_outA = dram.tile([N_CORES * P, P], bf16, tag="a2_outA")
            a2_inB = dram.tile([N_CORES * P, P], bf16, tag="a2_inB")
            a2_outB = dram.tile([N_CORES * P, P], bf16, tag="a2_outB")

            # pin the act table that covers BOTH Exp and Ln so the table-load
            # pass never swaps (it honors pre-placed loads)
            _tabs = list(get_activation_tables(nc.m.arch).keys())
            nc.scalar.add_instruction(mybir.InstLoadActFuncSet(
                name=nc.get_next_instruction_name(), ins=[], outs=[],
                act_func_set_id=_tabs.index("natural_log_exp_and_others")))

            ident_bf = pp.tile([P, P], bf16, tag="ident_bf")
            make_identity(nc, ident_bf[:])
            # additive causal mask for the diagonal block: 0 on/below diag,
            # -1e9 above; applied by PE as an accumulating matmul with the
            # identity as stationary (GPSIMD cannot touch PSUM)
            cmask = pp.tile([P, P], bf16, tag="cmask")
            nc.gpsimd.memset(cmask[:], 0.0)
            nc.gpsimd.affine_select(
                out=cmask[:], in_=cmask[:], compare_op=OP.is_ge, fill=-1e9,
                base=0, pattern=[[-1, P]], channel_multiplier=1)

            # persistent tensors
            qkT_all = pp.tile([P, 3, L], bf16, tag="qkT")
            qTs = [qkT_all[:, 0, :], qkT_all[:, 1, :]]
            kT = qkT_all[:, 2, :]       # kv head on both halves
            vm = pp.tile([P, NQB, D], bf16, tag="vm")  # v rows [l, d]
            lamneg_bc = pp.tile([P, 1], f32, tag="lamneg")
            # col u = 2*qb + pair
            rbuf1 = pp.tile([P, 32], f32, tag="rbuf1")
            rbuf2 = pp.tile([P, 32], f32, tag="rbuf2")
            ssqb = pp.tile([P, 32], f32, tag="ssqb")
            scl = pp.tile([P, 32], f32, tag="scl")
            out1_all = pp.tile([P, NQB, P], f32, tag="out1")  # [q, qb, 2x64]
            out1n = pp.tile([P, NQB, P], bf16, tag="out1n")
            onT = pp.tile([P, L], bf16, tag="onT")            # [dcat, L]

            xt_sb = pp.tile([P, NQB, KT, P], bf16, tag="xt")  # lg-major
            wqkv_sb = pp.tile([P, KT, 384], bf16, tag="wqkv")
            rope_sb = pp.tile([P, NQB, 64], f32, tag="rope")
            wo_sb = pp.tile([P, N_CORES, HID], bf16, tag="wo_sb")
            lam_sb = pp.tile([1, 1], f32, tag="lam_sb")
            sq_scr = pp.tile([P, D], f32, tag="sq_scr")
            omTA = pp.tile([P, N_CORES, P], bf16, tag="omTA")
            omTB = pp.tile([P, N_CORES, P], bf16, tag="omTB")

            # ---------------- DMA prologue ----------------
            # xt chunk 0 + the first wqkv quarter gate the first QKV matmul:
            # they go first; everything else trails.
            def emit_xt_dma(lg, eng):
                src = xtp[lg * 8 * P:(lg + 1) * 8 * P, :].rearrange(
                    "(k p) c -> p k c", p=P)
                dst = xt_sb[:, lg, :, :].rearrange("p k c -> p (k c)")
                dst = dst.rearrange("p (k c) -> p k c", k=8)
                eng.dma_start(dst, src)

            wqkv_r = wqkv[:].rearrange("(t p) c -> p t c", p=P)
            nc.scalar.dma_start(wqkv_sb[:, 0:4, :], wqkv_r[:, 0:4, :])

            def emit_xt_half(lg, half, eng):
                k0 = half * 4
                src_ = xtp[lg * 8 * P + k0 * P:lg * 8 * P + (k0 + 4) * P,
                           :].rearrange("(k p) c -> p k c", p=P)
                dst = xt_sb[:, lg, 2 * k0:2 * (k0 + 4), :].rearrange(
                    "p k c -> p (k c)").rearrange("p (k c) -> p k c", k=4)
                eng.dma_start(dst, src_)

            emit_xt_half(0, 0, nc.sync)
            emit_xt_half(0, 1, nc.sync)
            nc.scalar.dma_start(wqkv_sb[:, 4:8, :], wqkv_r[:, 4:8, :])
            emit_xt_half(1, 0, nc.sync)
            emit_xt_half(1, 1, nc.sync)
            nc.scalar.dma_start(wqkv_sb[:, 8:12, :], wqkv_r[:, 8:12, :])
            emit_xt_dma(2, nc.sync)
            nc.scalar.dma_start(wqkv_sb[:, 12:16, :], wqkv_r[:, 12:16, :])
            emit_xt_dma(3, nc.sync)
            nc.scalar.dma_start(lam_sb[:], lam_in[:])
            nc.sync.dma_start(
                rope_sb[:], ropet[:].rearrange("(g p) c -> p g c", p=P))
            for lg in range(4, NQB):
                emit_xt_dma(lg, nc.sync if lg % 2 == 0 else nc.scalar)
            wo_r = wo[:].rearrange("(c p) n -> p c n", p=P)
            nc.scalar.dma_start(wo_sb[:, 0:4, :], wo_r[:, 0:4, :])
            nc.scalar.dma_start(wo_sb[:, 4:8, :], wo_r[:, 4:8, :])

            nc.gpsimd.partition_broadcast(lamneg_bc[:], lam_sb[:])
            if debug:
                nc.sync.dma_start(dbg["d_lam"][:], lamneg_bc[:])

            # ---------------- fused main pipeline ----------------
            with (
                tc.tile_pool(name="pa2", bufs=2) as pa2,
                tc.tile_pool(name="pc2", bufs=2) as pc2,
                tc.tile_pool(name="psS", bufs=2, space="PSUM") as psS,
                tc.tile_pool(name="psT", bufs=1, space="PSUM") as psT,
            ):
                psQ_cm = tc.tile_pool(name="psQ", bufs=2, space="PSUM")
                psQ = psQ_cm.__enter__()

                pvx = psT.tile([P, 512], f32, tag="pvx")
                trn_t = psT.tile([P, 1024], bf16, tag="tr")
                tr_ctr = [0]

                def tr_half():
                    h = tr_ctr[0] % 2
                    tr_ctr[0] += 1
                    return trn_t[:, h * 512:(h + 1) * 512]

                def pv_buf(i):
                    return pvx[:, i * P:i * P + D]

                qkv_ps = {}

                def emit_qkv(lg):
                    ps = psQ.tile([P, 384], f32, tag="qkv", bufs=2,
                                  name=f"qkv{lg}")[:]
                    qkv_ps[lg] = ps
                    for kt in range(KT):
                        nc.tensor.matmul(
                            ps, xt_sb[:, lg, kt, :],
                            wqkv_sb[:, kt, :],
                            start=(kt == 0), stop=(kt == KT - 1))

                def emit_qkv01():
                    # kt-outer over the first two lgs so PE consumes the
                    # streaming wqkv quarters at half rate (no head stall)
                    qkv_ps[0] = psQ.tile([P, 384], f32, tag="qkv",
                                         bufs=2, name="qkv0")[:]
                    # lg1 borrows the (not-yet-used) pv bank as its
                    # accumulator during the kt-outer warmup
                    qkv_ps[1] = pvx[:, 0:384]
                    for kt in range(KT):
                        for lg in (0, 1):
                            nc.tensor.matmul(
                                qkv_ps[lg], xt_sb[:, lg, kt, :],
                                wqkv_sb[:, kt, :],
                                start=(kt == 0), stop=(kt == KT - 1))

                ropes = {}

                def emit_rope(lg):
                    # DVE stages qkv PSUM into SBUF; GPSIMD (Pool) does the
                    # rope math and the v copy, keeping ACT exp-only.
                    ps = qkv_ps.pop(lg)
                    stage = pa2.tile([P, 384], f32, tag="stg", bufs=2)
                    nc.vector.tensor_copy(stage[:], ps)
                    nc.gpsimd.tensor_copy(vm[:, lg, :], stage[:, 320:384])
                    qeng = nc.gpsimd
                    qsrc = stage
                    # ---- rope q: [128, 4h, 64] ----
                    qk_sb = pa2.tile([P, 384], bf16, tag="qk_sb", bufs=4)
                    q_sb = qk_sb[:, 0:256]
                    ta = pa2.tile([P, 4, 32], f32, tag="ta")
                    tb = pa2.tile([P, 4, 32], f32, tag="tb")
                    qp3 = qsrc[:, 0:256].rearrange("p (h j) -> p h j", j=D)
                    q3 = q_sb.rearrange("p (h j) -> p h j", j=D)
                    c3 = rope_sb[:, lg, 0:32].unsqueeze(1).broadcast_to([P, 4, 32])
                    s3 = rope_sb[:, lg, 32:64].unsqueeze(1).broadcast_to([P, 4, 32])
                    qeng.tensor_tensor(ta[:], qp3[:, :, 32:64], s3, OP.mult)
                    qeng.tensor_tensor(tb[:], qp3[:, :, 0:32], s3, OP.mult)
                    qeng.tensor_tensor(q3[:, :, 0:32], qp3[:, :, 0:32], c3, OP.mult)
                    qeng.tensor_tensor(q3[:, :, 32:64], qp3[:, :, 32:64], c3, OP.mult)
                    qeng.tensor_tensor(q3[:, :, 0:32], q3[:, :, 0:32], ta[:], OP.subtract)
                    qeng.tensor_tensor(q3[:, :, 32:64], q3[:, :, 32:64], tb[:], OP.add)
                    # ---- rope k: [128, 64] ----
                    k_sb = qk_sb[:, 256:320]
                    kc = rope_sb[:, lg, 0:32]
                    ks = rope_sb[:, lg, 32:64]
                    kta = pa2.tile([P, 32], f32, tag="kta")
                    ktb = pa2.tile([P, 32], f32, tag="ktb")
                    qeng.tensor_tensor(kta[:], qsrc[:, 288:320], ks, OP.mult)
                    qeng.tensor_tensor(ktb[:], qsrc[:, 256:288], ks, OP.mult)
                    qeng.tensor_tensor(k_sb[:, 0:32], qsrc[:, 256:288], kc, OP.mult)
                    qeng.tensor_tensor(k_sb[:, 32:64], qsrc[:, 288:320], kc, OP.mult)
                    qeng.tensor_tensor(k_sb[:, 0:32], k_sb[:, 0:32], kta[:], OP.subtract)
                    qeng.tensor_tensor(k_sb[:, 32:64], k_sb[:, 32:64], ktb[:], OP.add)
                    # duplicate k into the second half so the fused XBAR
                    # transpose lands [kT | kT] on the partition halves
                    qeng.tensor_copy(qk_sb[:, 320:384], k_sb)
                    ropes[lg] = qk_sb
                    if debug and lg == 0:
                        dq0 = pp.tile([P, 256], f32, tag="dq0")
                        nc.vector.tensor_copy(dq0[:], q_sb[:])
                        nc.sync.dma_start(dbg["d_q0"][:], dq0[:])
                        dk0 = pp.tile([P, D], f32, tag="dk0")
                        nc.vector.tensor_copy(dk0[:], k_sb)
                        nc.sync.dma_start(dbg["d_k0"][:], dk0[:])

                T_pend = {}

                def emit_T(lg):
                    qk_sb = ropes.pop(lg)
                    trp = tr_half()
                    for b in range(3):
                        nc.tensor.transpose(
                            trp[:, b * P:(b + 1) * P],
                            qk_sb[:, b * P:(b + 1) * P], ident_bf[:])
                    T_pend[lg] = trp

                def emit_T_copies(lg):
                    # deferred to the END of the iteration's DVE queue so
                    # DVE's dep-free work (diff/relu) runs first
                    trp = T_pend.pop(lg)
                    for b in range(3):
                        nc.vector.tensor_copy(
                            qkT_all[:, b, lg * P:(lg + 1) * P],
                            trp[:, b * P:(b + 1) * P])

                e_bufs = {}
                dT_bufs = {}
                pv_ctr = [0]

                def emit_front(pair, qb):
                    # scores + mask + exp (+rowsums) for one unit
                    qTp = qTs[pair]
                    u = 2 * qb + pair
                    span = (qb + 1) * P
                    e_b = pc2.tile([P, 2, L], bf16, tag="e", bufs=3)
                    for t in range(2):
                        rdst = rbuf1 if t == 0 else rbuf2
                        nchunk = (span + SCHUNK - 1) // SCHUNK
                        for ch in range(nchunk):
                            c0 = ch * SCHUNK
                            csp = min(SCHUNK, span - c0)
                            sps = psS.tile([P, SCHUNK], f32, tag="scores")
                            for m0 in range(0, csp, 512):
                                msp = min(512, csp - m0)
                                nc.tensor.matmul(
                                    sps[:, m0:m0 + msp],
                                    qTp[t * D:(t + 1) * D, qb * P:(qb + 1) * P],
                                    kT[t * D:(t + 1) * D, c0 + m0:c0 + m0 + msp],
                                    start=True, stop=True)
                            dlo = qb * P - c0
                            if 0 <= dlo < csp:
                                # causal mask: PE accumulates -1e9 above the
                                # diagonal (identity-stationary add)
                                nc.tensor.matmul(
                                    sps[:, dlo:dlo + P], ident_bf[:],
                                    cmask[:], start=False, stop=True,
                                    skip_group_check=True)
                            if ch == 0:
                                acc_ap = rdst[:, u:u + 1]
                            else:
                                rtmp = pc2.tile([P, 1], f32, tag="rtmp",
                                                bufs=4)
                                acc_ap = rtmp[:]
                            nc.scalar.activation(
                                e_b[:, t, c0:c0 + csp], sps[:, 0:csp],
                                AF.Exp, scale=SCALE, accum_out=acc_ap)
                            if ch > 0:
                                nc.gpsimd.tensor_tensor(
                                    rdst[:, u:u + 1], rdst[:, u:u + 1],
                                    rtmp[:], OP.add)
                    e_bufs[(pair, qb)] = e_b

                def emit_back(pair, qb):
                    # fused diff + transpose + PV for one unit
                    e_b = e_bufs.pop((pair, qb))
                    u = 2 * qb + pair
                    span = (qb + 1) * P
                    # lam_pr = -lam * r1 / r2, then diff = e1 + lam_pr*e2 in
                    # a single scalar_tensor_tensor
                    lam_p = pc2.tile([P, 1], f32, tag="lam_p")
                    nc.vector.reciprocal(lam_p[:], rbuf2[:, u:u + 1])
                    nc.vector.tensor_tensor(
                        lam_p[:], lam_p[:], lamneg_bc[:], OP.mult)
                    nc.vector.tensor_tensor(
                        lam_p[:], lam_p[:], rbuf1[:, u:u + 1], OP.mult)
                    diff = pc2.tile([P, L], bf16, tag="diff", bufs=4)
                    nc.vector.scalar_tensor_tensor(
                        out=diff[:, 0:span], in0=e_b[:, 1, 0:span],
                        scalar=lam_p[:], in1=e_b[:, 0, 0:span],
                        op0=OP.mult, op1=OP.add)
                    if debug and pair == 0 and qb == 3:
                        de = pc2.tile([P, L], f32, tag="de")
                        nc.vector.tensor_copy(de[:, 0:span], e_b[:, 0, 0:span])
                        nc.sync.dma_start(dbg["d_e0"][:], de[:])
                        nc.vector.tensor_copy(de[:, 0:span], diff[:, 0:span])
                        nc.sync.dma_start(dbg["d_diff3"][:], de[:])
                    dT_bufs[(pair, qb)] = diff

                def emit_pv(pair, qb):
                    # PE transposes group g while DVE relu-copies g-1 and PE
                    # PVs g-1 (software-pipelined); trails the diff by an
                    # iteration so DVE's queue never stalls PE
                    diff = dT_bufs.pop((pair, qb))
                    u = 2 * qb + pair
                    span = (qb + 1) * P
                    pvh = pv_ctr[0] % 2
                    pv_ctr[0] += 1
                    pv = pv_buf(pvh)
                    nkb = qb + 1
                    grps = []

                    def rpv(g):
                        kb0, ng, trp = g
                        dT = pc2.tile([P, 512], bf16, tag="dT", bufs=3)
                        nc.vector.tensor_scalar(
                            out=dT[:, 0:ng * P], in0=trp[:, 0:ng * P],
                            scalar1=0.0, scalar2=None, op0=OP.max)
                        for i in range(ng):
                            kb = kb0 + i
                            nc.tensor.matmul(
                                pv[:], dT[:, i * P:(i + 1) * P],
                                vm[:, kb, :],
                                start=(kb == 0), stop=(kb == nkb - 1))

                    for grp in range((nkb + 3) // 4):
                        kb0 = grp * 4
                        ng = min(4, nkb - kb0)
                        trp = tr_half()
                        for i in range(ng):
                            nc.tensor.transpose(
                                trp[:, i * P:(i + 1) * P],
                                diff[:, (kb0 + i) * P:(kb0 + i + 1) * P],
                                ident_bf[:])
                        grps.append((kb0, ng, trp))
                        if len(grps) >= 2:
                            rpv(grps.pop(0))
                    while grps:
                        rpv(grps.pop(0))
                    # stash out1 (DVE), then ssq from the SBUF copy
                    o1 = out1_all[:, qb, pair * D:(pair + 1) * D]
                    nc.vector.tensor_copy(o1, pv[:])
                    nc.vector.scalar_tensor_tensor(
                        out=sq_scr[:], in0=o1, scalar=1.0, in1=o1,
                        op0=OP.mult, op1=OP.mult,
                        accum_out=ssqb[:, u:u + 1])

                def _rms_pair(qb0, nqb2):
                    # rsqrt via ln+exp for qbs [qb0, qb0+nqb2) (cols 2*qb0..)
                    u0 = 2 * qb0
                    w = 2 * nqb2
                    rsq = pc2.tile([P, 4], f32, tag="rsq")
                    nc.gpsimd.tensor_tensor(
                        rsq[:, 0:w], rbuf1[:, u0:u0 + w], rbuf1[:, u0:u0 + w],
                        OP.mult)
                    uarg = pc2.tile([P, 4], f32, tag="uarg")
                    nc.vector.scalar_tensor_tensor(
                        out=uarg[:, 0:w], in0=rsq[:, 0:w],
                        scalar=float(D) * 1e-6,
                        in1=ssqb[:, u0:u0 + w], op0=OP.mult, op1=OP.add)
                    lnu = pc2.tile([P, 4], f32, tag="lnu")
                    nc.scalar.activation(lnu[:, 0:w], uarg[:, 0:w], AF.Ln,
                                         scale=1.0 / D)
                    nc.scalar.activation(scl[:, u0:u0 + w], lnu[:, 0:w],
                                         AF.Exp, scale=-0.5)

                def emit_qbdone(qb):
                    # batched: odd qb handles {qb-1, qb}
                    if qb % 2 == 0:
                        return
                    _rms_pair(qb - 1, 2)
                    for qbx in (qb - 1, qb):
                        u0 = 2 * qbx
                        for pair in range(2):
                            sl = slice(pair * D, (pair + 1) * D)
                            nc.vector.tensor_scalar(
                                out=out1n[:, qbx, sl],
                                in0=out1_all[:, qbx, sl],
                                scalar1=scl[:, u0 + pair:u0 + pair + 1],
                                scalar2=None, op0=OP.mult)
                        if qbx < 14:
                            nc.sync.dma_start_transpose(
                                onT[:, qbx * P:(qbx + 1) * P],
                                out1n[:, qbx, :])
                        else:
                            trp = tr_half()
                            nc.tensor.transpose(
                                trp[:, 0:P], out1n[:, qbx, :], ident_bf[:])
                            nc.scalar.copy(onT[:, qbx * P:(qbx + 1) * P],
                                           trp[:, 0:P])
                        tgt = a2_inA if qbx < 8 else a2_inB
                        blk = qbx % 8
                        nc.sync.dma_start(
                            tgt[blk * P:(blk + 1) * P, :],
                            onT[:, qbx * P:(qbx + 1) * P])

                def emit_a2a(which):
                    a2i = a2_inA if which == 0 else a2_inB
                    a2o = a2_outA if which == 0 else a2_outB
                    if mock_collectives:
                        nc.sync.dma_start(a2o[:], a2i[:])
                    else:
                        nc.gpsimd.collective_compute(
                            "AllToAll", OP.bypass,
                            replica_groups=[list(range(N_CORES))],
                            ins=[a2i.opt()], outs=[a2o.opt()])

                # ---- the pipeline ----
                # per iteration lg: qkv(lg+1), T(lg-1), qbdone(lg-4),
                # rope(lg), fronts(lg-1), diffs(lg-2), pvs(lg-3)
                pv_done = set()
                qb_done = set()

                def maybe_pv(qb):
                    if 0 <= qb < NQB and qb not in pv_done:
                        pv_done.add(qb)
                        emit_pv(0, qb)
                        emit_pv(1, qb)

                def maybe_qbdone(qb):
                    if 0 <= qb < NQB and qb not in qb_done:
                        qb_done.add(qb)
                        emit_qbdone(qb)

                for lg in range(NQB):
                    if lg == 0:
                        emit_qkv01()
                    elif lg < NQB - 1:
                        emit_qkv(lg + 1)
                    if lg >= 1:
                        emit_T(lg - 1)
                    # early qbs finish shallow so A2A#1 fires at lg 10 and
                    # the Wo first half hoists into the ACT-paced mid-body;
                    # late qbs keep the deep trailing that hides DVE latency
                    if lg - 3 <= 7:
                        maybe_qbdone(lg - 3)
                    maybe_qbdone(lg - 6)
                    if lg == 10:
                        emit_a2a(0)
                        nc.scalar.dma_start(
                            omTA[:],
                            a2_outA[:].rearrange("(c p) l -> p c l", p=P))
                    emit_rope(lg)
                    if lg >= 1:
                        emit_T_copies(lg - 1)
                    if lg >= 1:
                        emit_front(0, lg - 1)
                    if lg >= 2:
                        emit_back(0, lg - 2)
                    if lg >= 1:
                        emit_front(1, lg - 1)
                    if lg >= 2:
                        emit_back(1, lg - 2)
                    if lg - 2 <= 7:
                        maybe_pv(lg - 2)
                    maybe_pv(lg - 5)
                psQ_cm.__exit__(None, None, None)

                emit_T(NQB - 1)
                emit_T_copies(NQB - 1)
                emit_front(0, NQB - 1)
                emit_back(0, NQB - 2)
                maybe_pv(NQB - 5)
                emit_front(1, NQB - 1)
                emit_back(1, NQB - 2)
                maybe_qbdone(NQB - 6)
                emit_back(0, NQB - 1)
                maybe_pv(NQB - 4)
                emit_back(1, NQB - 1)
                maybe_qbdone(NQB - 5)
                maybe_pv(NQB - 3)
                maybe_qbdone(NQB - 4)
                maybe_pv(NQB - 2)
                maybe_qbdone(NQB - 3)
                maybe_pv(NQB - 1)
                maybe_qbdone(NQB - 2)
                maybe_qbdone(NQB - 1)
                emit_a2a(1)
                nc.scalar.dma_start(
                    omTB[:], a2_outB[:].rearrange("(c p) l -> p c l", p=P))

                # ---------------- Wo (A overlaps the second A2A) ----------
                with tc.tile_pool(name="psD", bufs=2, space="PSUM") as psD:
                    def emit_wo(half, omT):
                        for n4 in range(4):
                            csl = slice(n4 * 512, (n4 + 1) * 512)
                            ops = psD.tile([P, 512], f32, tag="ops", bufs=2)
                            for dchunk in range(N_CORES):
                                nc.tensor.matmul(
                                    ops[:], omT[:, dchunk, :],
                                    wo_sb[:, dchunk, csl],
                                    start=(dchunk == 0),
                                    stop=(dchunk == N_CORES - 1))
                            o_sb = pc2.tile([P, 512], bf16, tag="o_sb",
                                            bufs=2)
                            nc.scalar.copy(o_sb[:], ops[:])
                            nc.sync.dma_start(
                                out_d[half * P:(half + 1) * P, csl], o_sb[:])

                    emit_wo(0, omTA)
                    emit_wo(1, omTB)

                if debug:
                    dqt = pp.tile([P, L], f32, tag="dqt")
                    nc.vector.tensor_copy(dqt[:], qTs[0][:])
                    nc.sync.dma_start(dbg["d_qT0"][:], dqt[:])
                    nc.vector.tensor_copy(dqt[:], kT[:])
                    nc.sync.dma_start(dbg["d_kT"][:], dqt[:])
                    dvm = pp.tile([P, D], f32, tag="dvm")
                    nc.vector.tensor_copy(dvm[:], vm[:, 3, :])
                    nc.sync.dma_start(dbg["d_vm"][:], dvm[:])
                    nc.sync.dma_start(dbg["d_r1"][:], rbuf1[:])
                    nc.sync.dma_start(dbg["d_r2"][:], rbuf2[:])
                    nc.sync.dma_start(dbg["d_ssq"][:], ssqb[:])
                    nc.sync.dma_start(dbg["d_scl"][:], scl[:])
                    nc.sync.dma_start(dbg["d_out1"][:], out1_all[:])
                    nc.vector.tensor_copy(dqt[:], onT[:])
                    nc.sync.dma_start(dbg["d_onT"][:], dqt[:])

    return nc


_CACHE = {}


def _get_program():
    if "nc" not in _CACHE:
        nc = _build()
        nc.compile()
        _CACHE["nc"] = nc
    return _CACHE["nc"]


def _host_lambda(x2, cos, sin, Wq, Wk, lq1, lk1, lq2, lk2):
    """Exact lambda: the dots are linear+rope functionals of x."""
    c32 = cos[:L, :32]
    s32 = sin[:L, :32]

    def rt(lam):
        l1, l2 = lam[:32], lam[32:]
        return np.concatenate([c32 * l1 + s32 * l2, -s32 * l1 + c32 * l2],
                              axis=1)  # [L, 64] rows R_l^T lam

    Wq3 = Wq.reshape(HID, H, D)
    Wqe = Wq3[:, 0::2, :].sum(axis=1)
    Wqo = Wq3[:, 1::2, :].sum(axis=1)
    Wks = Wk.reshape(HID, HKV, D).sum(axis=1)
    pk = x2 @ Wks
    d1 = np.clip(((x2 @ Wqe) * rt(lq1)).sum() / L, -10.0, 10.0)
    d2 = np.clip(2.0 * (pk * rt(lk1)).sum() / L, -10.0, 10.0)
    d3 = np.clip(((x2 @ Wqo) * rt(lq2)).sum() / L, -10.0, 10.0)
    d4 = np.clip(2.0 * (pk * rt(lk2)).sum() / L, -10.0, 10.0)
    lam = np.exp(d1) * np.exp(d2) - np.exp(d3) * np.exp(d4) + LAMBDA_INIT
    return float(np.clip(lam, 0.0, 1.0))


def _host_prep(x, cos, sin, Wq, Wk, Wv, Wo, lambda_q1, lambda_k1, lambda_q2,
               lambda_k2, subln_weight):
    bf = ml_dtypes.bfloat16
    x2 = np.asarray(x, np.float32).reshape(L, HID)
    cos = np.asarray(cos, np.float32)
    sin = np.asarray(sin, np.float32)
    # xt swizzled for 512B-contiguous lg-major streaming:
    # xtp[lg, ktp, p, e*128+col] = x2[lg*128+col, (2*ktp+e)*128+p]
    xT = np.ascontiguousarray(x2.T)                      # [HID, L]
    Bv = xT.reshape(8, 2, P, NQB, P)                     # [ktp, e, p, lg, col]
    xtp = np.ascontiguousarray(
        Bv.transpose(3, 0, 2, 1, 4).reshape(NQB * 8 * P, 256)).astype(bf)
    ropet = np.ascontiguousarray(
        np.concatenate([cos[:L, :32], sin[:L, :32]], axis=1))  # [L, 64]
    Wq = np.asarray(Wq, np.float32)
    Wk = np.asarray(Wk, np.float32)
    Wv = np.asarray(Wv, np.float32)
    lq1 = np.asarray(lambda_q1, np.float32)
    lq2 = np.asarray(lambda_q2, np.float32)
    lk1 = np.asarray(lambda_k1, np.float32)
    lk2 = np.asarray(lambda_k2, np.float32)
    lam = _host_lambda(x2, cos, sin, Wq, Wk, lq1, lk1, lq2, lk2)
    lamneg = np.full((1, 1), -lam, np.float32)
    s = np.asarray(subln_weight, np.float32) * (1.0 - LAMBDA_INIT)   # [128]
    Wo = np.asarray(Wo, np.float32)
    wo_eff = np.empty((H * D // 2, HID), np.float32)
    for p in range(H // 2):
        blk = Wo[p * 2 * D:(p + 1) * 2 * D, :]           # [128, HID]
        wo_eff[p * D:(p + 1) * D] = (s[:D, None] * blk[:D]
                                     + s[D:, None] * blk[D:])
    wo_eff = wo_eff.astype(bf)
    in_maps = []
    for c in range(N_CORES):
        wqkv_c = np.concatenate([
            Wq[:, c * NH * D:(c + 1) * NH * D],
            Wk[:, c * D:(c + 1) * D],
            Wv[:, c * D:(c + 1) * D]], axis=1).astype(bf)    # [HID, 384]
        in_maps.append({
            "xtp": xtp, "wqkv": wqkv_c, "ropet": ropet, "lamneg": lamneg,
            "wo": wo_eff,
        })
    return in_maps


def kernel(**inputs) -> np.ndarray:
    nc = _get_program()
    in_maps = _host_prep(**{k: v for k, v in inputs.items() if k != "mask"})
    res = run_bass_kernel_spmd(nc, in_maps, list(range(N_CORES)))
    out = np.empty((L, HID), np.float32)
    for c in range(N_CORES):
        r = np.asarray(res.results[c]["out"], np.float32)
        out[c * P:(c + 1) * P] = r[0:P]
        out[(c + 8) * P:(c + 9) * P] = r[P:2 * P]
    return out.reshape(1, L, HID)


# revision 5
# speedup vs baseline: 1.0411x; 1.0411x over previous
"""Trainium2 Bass kernel for nn_DifferentialGQA (8-core SPMD), v3.

Fully-fused single-phase pipeline (tensor-parallel, no mid-kernel AllReduce):
  - lambda is computed EXACTLY on the host (it is a linear+rope functional of
    x and the weights), so the AllReduce and the phase-A/phase-C serialization
    of v2 disappear entirely.
  - Per 128-row block lg: QKV matmuls (PE) -> rope (DVE stage + GPSIMD) ->
    q/k transposes (PE) -> attention fronts for qb=lg (score matmuls +
    diag-mask matmul; ACT exp with free row-sum accumulation) -> backs
    trailing one lg (fused diff stt on DVE, PE transposes + PV) -> per-qb RMS
    + output transpose trailing three lgs. ACT's exp stream overlaps the QKV
    GEMMs; PE stays continuously busy (p-state friendly).
  - Engine budget: PE ~117us (the roofline), ACT ~97us (exp+accum), DVE ~85us
    (copies + fused diff), Pool ~65us (ropes + small elementwise).
  - x^T is host-swizzled so each lg-major stream chunk is 512B-contiguous
    (full DMA rate); PE starts after ~6us.
  - Output reshard uses TWO AllToAlls over an interleaved qb->core map
    (core c owns L rows of q-blocks c and c+8): the first fires mid-kernel
    (fully hidden), the second at the end overlaps the first Wo half.
  - RMS rsqrt via ln+exp (both live in the natural_log_exp activation table:
    zero table swaps).
"""
import sys

sys.path.insert(0, "/opt/trn_rl_repo")

import numpy as np
import ml_dtypes

import concourse.bass as bass
import concourse.mybir as mybir
import concourse.tile as tile
from concourse import bacc
from concourse.bass_utils import run_bass_kernel_spmd
from concourse.hw_specs import get_activation_tables
from concourse.masks import make_identity

dt = mybir.dt
AF = mybir.ActivationFunctionType
OP = mybir.AluOpType

N_CORES = 8
L = 2048
HID = 2048
H = 32
HKV = 8
D = 64
CAP = 50.0
LAMBDA_INIT = 0.8 - 0.6 * float(np.exp(-0.3 * 4))
P = 128
LROWS = L // N_CORES          # 256 output rows per core
NQB = L // P                  # 16 query blocks
KT = HID // P                 # 16 contraction tiles
NH = H // N_CORES             # 4 q heads per core
SCALE = 1.0 / float(np.sqrt(D))
SCHUNK = 1024                 # exp chunk (2 PSUM banks of f32)


def _build(mock_collectives: bool = False, debug: bool = False):
    nc = bacc.Bacc("TRN2", target_bir_lowering=False, debug=False,
                   num_devices=(1 if mock_collectives else N_CORES))
    f32, bf16 = dt.float32, dt.bfloat16

    xtp = nc.dram_tensor("xtp", [NQB * 8 * P, 256], bf16,
                         kind="ExternalInput").ap()
    wqkv = nc.dram_tensor("wqkv", [HID, 384], bf16, kind="ExternalInput").ap()
    ropet = nc.dram_tensor("ropet", [L, 64], f32, kind="ExternalInput").ap()
    lam_in = nc.dram_tensor("lamneg", [1, 1], f32, kind="ExternalInput").ap()
    wo = nc.dram_tensor("wo", [H * D // 2, HID], bf16, kind="ExternalInput").ap()
    out_d = nc.dram_tensor("out", [LROWS, HID], bf16, kind="ExternalOutput").ap()
    dbg = {}
    if debug:
        for nm, shp, dty in [
            ("d_q0", [P, 256], f32), ("d_k0", [P, D], f32),
            ("d_qT0", [P, L], f32), ("d_kT", [P, L], f32),
            ("d_vm", [P, D], f32), ("d_lam", [P, 1], f32),
            ("d_r1", [P, 32], f32), ("d_r2", [P, 32], f32),
            ("d_e0", [P, L], f32), ("d_diff3", [P, L], f32),
            ("d_out1", [P, NQB, P], f32), ("d_ssq", [P, 32], f32),
            ("d_scl", [P, 32], f32), ("d_onT", [P, L], f32),
        ]:
            dbg[nm] = nc.dram_tensor(nm, shp, dty, kind="ExternalOutput").ap()

    with tile.TileContext(nc) as tc:
        with (
            tc.tile_pool(name="persist", bufs=1) as pp,
            tc.tile_pool(name="dram", bufs=1, space="DRAM") as dram,
        ):
            a2_inA = dram.tile([N_CORES * P, P], bf16, tag="a2_inA")
            a2_outA = dram.tile([N_CORES * P, P], bf16, tag="a2_outA")
            a2_inB = dram.tile([N_CORES * P, P], bf16, tag="a2_inB")
            a2_outB = dram.tile([N_CORES * P, P], bf16, tag="a2_outB")

            # pin the act table that covers BOTH Exp and Ln so the table-load
            # pass never swaps (it honors pre-placed loads)
            _tabs = list(get_activation_tables(nc.m.arch).keys())
            nc.scalar.add_instruction(mybir.InstLoadActFuncSet(
                name=nc.get_next_instruction_name(), ins=[], outs=[],
                act_func_set_id=_tabs.index("natural_log_exp_and_others")))

            ident_bf = pp.tile([P, P], bf16, tag="ident_bf")
            make_identity(nc, ident_bf[:])
            # additive causal mask for the diagonal block: 0 on/below diag,
            # -1e9 above; applied by PE as an accumulating matmul with the
            # identity as stationary (GPSIMD cannot touch PSUM)
            cmask = pp.tile([P, P], bf16, tag="cmask")
            nc.gpsimd.memset(cmask[:], 0.0)
            nc.gpsimd.affine_select(
                out=cmask[:], in_=cmask[:], compare_op=OP.is_ge, fill=-1e9,
                base=0, pattern=[[-1, P]], channel_multiplier=1)

            # persistent tensors
            qkT_all = pp.tile([P, 3, L], bf16, tag="qkT")
            qTs = [qkT_all[:, 0, :], qkT_all[:, 1, :]]
            kT = qkT_all[:, 2, :]       # kv head on both halves
            vm = pp.tile([P, NQB, D], bf16, tag="vm")  # v rows [l, d]
            lamneg_bc = pp.tile([P, 1], f32, tag="lamneg")
            # col u = 2*qb + pair
            rbuf1 = pp.tile([P, 32], f32, tag="rbuf1")
            rbuf2 = pp.tile([P, 32], f32, tag="rbuf2")
            ssqb = pp.tile([P, 32], f32, tag="ssqb")
            scl = pp.tile([P, 32], f32, tag="scl")
            out1_all = pp.tile([P, NQB, P], f32, tag="out1")  # [q, qb, 2x64]
            out1n = pp.tile([P, NQB, P], bf16, tag="out1n")
            onT = pp.tile([P, L], bf16, tag="onT")            # [dcat, L]

            xt_sb = pp.tile([P, NQB, KT, P], bf16, tag="xt")  # lg-major
            wqkv_sb = pp.tile([P, KT, 384], bf16, tag="wqkv")
            rope_sb = pp.tile([P, NQB, 64], f32, tag="rope")
            wo_sb = pp.tile([P, N_CORES, HID], bf16, tag="wo_sb")
            lam_sb = pp.tile([1, 1], f32, tag="lam_sb")
            sq_scr = pp.tile([P, D], f32, tag="sq_scr")
            omTA = pp.tile([P, N_CORES, P], bf16, tag="omTA")
            omTB = pp.tile([P, N_CORES, P], bf16, tag="omTB")

            # ---------------- DMA prologue ----------------
            # xt chunk 0 + the first wqkv quarter gate the first QKV matmul:
            # they go first; everything else trails.
            def emit_xt_dma(lg, eng):
                src = xtp[lg * 8 * P:(lg + 1) * 8 * P, :].rearrange(
                    "(k p) c -> p k c", p=P)
                dst = xt_sb[:, lg, :, :].rearrange("p k c -> p (k c)")
                dst = dst.rearrange("p (k c) -> p k c", k=8)
                eng.dma_start(dst, src)

            wqkv_r = wqkv[:].rearrange("(t p) c -> p t c", p=P)
            nc.scalar.dma_start(wqkv_sb[:, 0:4, :], wqkv_r[:, 0:4, :])

            def emit_xt_half(lg, half, eng):
                k0 = half * 4
                src_ = xtp[lg * 8 * P + k0 * P:lg * 8 * P + (k0 + 4) * P,
                           :].rearrange("(k p) c -> p k c", p=P)
                dst = xt_sb[:, lg, 2 * k0:2 * (k0 + 4), :].rearrange(
                    "p k c -> p (k c)").rearrange("p (k c) -> p k c", k=4)
                eng.dma_start(dst, src_)

            emit_xt_half(0, 0, nc.sync)
            emit_xt_half(0, 1, nc.sync)
            nc.scalar.dma_start(wqkv_sb[:, 4:8, :], wqkv_r[:, 4:8, :])
            emit_xt_half(1, 0, nc.sync)
            emit_xt_half(1, 1, nc.sync)
            nc.scalar.dma_start(wqkv_sb[:, 8:12, :], wqkv_r[:, 8:12, :])
            emit_xt_dma(2, nc.sync)
            nc.scalar.dma_start(wqkv_sb[:, 12:16, :], wqkv_r[:, 12:16, :])
            emit_xt_dma(3, nc.sync)
            nc.scalar.dma_start(lam_sb[:], lam_in[:])
            nc.sync.dma_start(
                rope_sb[:], ropet[:].rearrange("(g p) c -> p g c", p=P))
            for lg in range(4, NQB):
                emit_xt_dma(lg, nc.sync if lg % 2 == 0 else nc.scalar)
            wo_r = wo[:].rearrange("(c p) n -> p c n", p=P)
            nc.scalar.dma_start(wo_sb[:, 0:4, :], wo_r[:, 0:4, :])
            nc.scalar.dma_start(wo_sb[:, 4:8, :], wo_r[:, 4:8, :])

            nc.gpsimd.partition_broadcast(lamneg_bc[:], lam_sb[:])
            if debug:
                nc.sync.dma_start(dbg["d_lam"][:], lamneg_bc[:])

            # ---------------- fused main pipeline ----------------
            with (
                tc.tile_pool(name="pa2", bufs=2) as pa2,
                tc.tile_pool(name="pc2", bufs=2) as pc2,
                tc.tile_pool(name="psS", bufs=2, space="PSUM") as psS,
                tc.tile_pool(name="psT", bufs=1, space="PSUM") as psT,
            ):
                psQ_cm = tc.tile_pool(name="psQ", bufs=2, space="PSUM")
                psQ = psQ_cm.__enter__()

                pvx = psT.tile([P, 512], f32, tag="pvx")
                trn_t = psT.tile([P, 1024], bf16, tag="tr")
                tr_ctr = [0]

                def tr_half():
                    h = tr_ctr[0] % 2
                    tr_ctr[0] += 1
                    return trn_t[:, h * 512:(h + 1) * 512]

                def pv_buf(i):
                    return pvx[:, i * P:i * P + D]

                qkv_ps = {}

                def emit_qkv(lg):
                    ps = psQ.tile([P, 384], f32, tag="qkv", bufs=2,
                                  name=f"qkv{lg}")[:]
                    qkv_ps[lg] = ps
                    for kt in range(KT):
                        nc.tensor.matmul(
                            ps, xt_sb[:, lg, kt, :],
                            wqkv_sb[:, kt, :],
                            start=(kt == 0), stop=(kt == KT - 1))

                def emit_qkv01():
                    # kt-outer over the first two lgs so PE consumes the
                    # streaming wqkv quarters at half rate (no head stall)
                    qkv_ps[0] = psQ.tile([P, 384], f32, tag="qkv",
                                         bufs=2, name="qkv0")[:]
                    # lg1 borrows the (not-yet-used) pv bank as its
                    # accumulator during the kt-outer warmup
                    qkv_ps[1] = pvx[:, 0:384]
                    for kt in range(KT):
                        for lg in (0, 1):
                            nc.tensor.matmul(
                                qkv_ps[lg], xt_sb[:, lg, kt, :],
                                wqkv_sb[:, kt, :],
                                start=(kt == 0), stop=(kt == KT - 1))

                ropes = {}

                def emit_rope(lg):
                    # DVE stages qkv PSUM into SBUF; GPSIMD (Pool) does the
                    # rope math and the v copy, keeping ACT exp-only. The
                    # first two ropes go straight through DVE from PSUM (DVE
                    # is idle at the head and the stage+Pool chain would
                    # delay T(0)/T(1) by ~1.5us each).
                    ps = qkv_ps.pop(lg)
                    if lg < 3:
                        nc.vector.tensor_copy(vm[:, lg, :], ps[:, 320:384])
                        qeng = nc.vector
                        qsrc = ps
                    else:
                        stage = pa2.tile([P, 384], f32, tag="stg", bufs=2)
                        nc.vector.tensor_copy(stage[:], ps)
                        nc.gpsimd.tensor_copy(vm[:, lg, :], stage[:, 320:384])
                        qeng = nc.gpsimd
                        qsrc = stage
                    # ---- rope q: [128, 4h, 64] ----
                    qk_sb = pa2.tile([P, 384], bf16, tag="qk_sb", bufs=4)
                    q_sb = qk_sb[:, 0:256]
                    ta = pa2.tile([P, 4, 32], f32, tag="ta")
                    tb = pa2.tile([P, 4, 32], f32, tag="tb")
                    qp3 = qsrc[:, 0:256].rearrange("p (h j) -> p h j", j=D)
                    q3 = q_sb.rearrange("p (h j) -> p h j", j=D)
                    c3 = rope_sb[:, lg, 0:32].unsqueeze(1).broadcast_to([P, 4, 32])
                    s3 = rope_sb[:, lg, 32:64].unsqueeze(1).broadcast_to([P, 4, 32])
                    qeng.tensor_tensor(ta[:], qp3[:, :, 32:64], s3, OP.mult)
                    qeng.tensor_tensor(tb[:], qp3[:, :, 0:32], s3, OP.mult)
                    qeng.tensor_tensor(q3[:, :, 0:32], qp3[:, :, 0:32], c3, OP.mult)
                    qeng.tensor_tensor(q3[:, :, 32:64], qp3[:, :, 32:64], c3, OP.mult)
                    qeng.tensor_tensor(q3[:, :, 0:32], q3[:, :, 0:32], ta[:], OP.subtract)
                    qeng.tensor_tensor(q3[:, :, 32:64], q3[:, :, 32:64], tb[:], OP.add)
                    # ---- rope k: [128, 64] ----
                    k_sb = qk_sb[:, 256:320]
                    kc = rope_sb[:, lg, 0:32]
                    ks = rope_sb[:, lg, 32:64]
                    kta = pa2.tile([P, 32], f32, tag="kta")
                    ktb = pa2.tile([P, 32], f32, tag="ktb")
                    qeng.tensor_tensor(kta[:], qsrc[:, 288:320], ks, OP.mult)
                    qeng.tensor_tensor(ktb[:], qsrc[:, 256:288], ks, OP.mult)
                    qeng.tensor_tensor(k_sb[:, 0:32], qsrc[:, 256:288], kc, OP.mult)
                    qeng.tensor_tensor(k_sb[:, 32:64], qsrc[:, 288:320], kc, OP.mult)
                    qeng.tensor_tensor(k_sb[:, 0:32], k_sb[:, 0:32], kta[:], OP.subtract)
                    qeng.tensor_tensor(k_sb[:, 32:64], k_sb[:, 32:64], ktb[:], OP.add)
                    # duplicate k into the second half so the fused XBAR
                    # transpose lands [kT | kT] on the partition halves
                    qeng.tensor_copy(qk_sb[:, 320:384], k_sb)
                    ropes[lg] = qk_sb
                    if debug and lg == 0:
                        dq0 = pp.tile([P, 256], f32, tag="dq0")
                        nc.vector.tensor_copy(dq0[:], q_sb[:])
                        nc.sync.dma_start(dbg["d_q0"][:], dq0[:])
                        dk0 = pp.tile([P, D], f32, tag="dk0")
                        nc.vector.tensor_copy(dk0[:], k_sb)
                        nc.sync.dma_start(dbg["d_k0"][:], dk0[:])

                T_pend = {}

                def emit_T(lg):
                    qk_sb = ropes.pop(lg)
                    trp = tr_half()
                    for b in range(3):
                        nc.tensor.transpose(
                            trp[:, b * P:(b + 1) * P],
                            qk_sb[:, b * P:(b + 1) * P], ident_bf[:])
                    T_pend[lg] = trp

                def emit_T_copies(lg):
                    # deferred to the END of the iteration's DVE queue so
                    # DVE's dep-free work (diff/relu) runs first
                    trp = T_pend.pop(lg)
                    for b in range(3):
                        nc.vector.tensor_copy(
                            qkT_all[:, b, lg * P:(lg + 1) * P],
                            trp[:, b * P:(b + 1) * P])

                e_bufs = {}
                dT_bufs = {}
                pv_ctr = [0]

                def emit_front(pair, qb):
                    # scores + mask + exp (+rowsums) for one unit
                    qTp = qTs[pair]
                    u = 2 * qb + pair
                    span = (qb + 1) * P
                    e_b = pc2.tile([P, 2, L], bf16, tag="e", bufs=3)
                    for t in range(2):
                        rdst = rbuf1 if t == 0 else rbuf2
                        nchunk = (span + SCHUNK - 1) // SCHUNK
                        # small remainder chunk FIRST so the unit's first exp
                        # fires after less PE work (earlier ACT start)
                        bounds = [0, span - SCHUNK] if nchunk == 2 else [0]
                        for ch, c0 in enumerate(bounds):
                            csp = (span - SCHUNK if nchunk == 2 and ch == 0
                                   else min(SCHUNK, span - c0))
                            sps = psS.tile([P, SCHUNK], f32, tag="scores")
                            for m0 in range(0, csp, 512):
                                msp = min(512, csp - m0)
                                nc.tensor.matmul(
                                    sps[:, m0:m0 + msp],
                                    qTp[t * D:(t + 1) * D, qb * P:(qb + 1) * P],
                                    kT[t * D:(t + 1) * D, c0 + m0:c0 + m0 + msp],
                                    start=True, stop=True)
                            dlo = qb * P - c0
                            if 0 <= dlo < csp:
                                # causal mask: PE accumulates -1e9 above the
                                # diagonal (identity-stationary add)
                                nc.tensor.matmul(
                                    sps[:, dlo:dlo + P], ident_bf[:],
                                    cmask[:], start=False, stop=True,
                                    skip_group_check=True)
                            if ch == 0:
                                acc_ap = rdst[:, u:u + 1]
                            else:
                                rtmp = pc2.tile([P, 1], f32, tag="rtmp",
                                                bufs=4)
                                acc_ap = rtmp[:]
                            nc.scalar.activation(
                                e_b[:, t, c0:c0 + csp], sps[:, 0:csp],
                                AF.Exp, scale=SCALE, accum_out=acc_ap)
                            if ch > 0:
                                nc.gpsimd.tensor_tensor(
                                    rdst[:, u:u + 1], rdst[:, u:u + 1],
                                    rtmp[:], OP.add)
                    e_bufs[(pair, qb)] = e_b

                def emit_back(pair, qb):
                    # fused diff + transpose + PV for one unit
                    e_b = e_bufs.pop((pair, qb))
                    u = 2 * qb + pair
                    span = (qb + 1) * P
                    # lam_pr = -lam * r1 / r2, then diff = e1 + lam_pr*e2 in
                    # a single scalar_tensor_tensor
                    lam_p = pc2.tile([P, 1], f32, tag="lam_p")
                    nc.vector.reciprocal(lam_p[:], rbuf2[:, u:u + 1])
                    nc.vector.tensor_tensor(
                        lam_p[:], lam_p[:], lamneg_bc[:], OP.mult)
                    nc.vector.tensor_tensor(
                        lam_p[:], lam_p[:], rbuf1[:, u:u + 1], OP.mult)
                    diff = pc2.tile([P, L], bf16, tag="diff", bufs=4)
                    nc.vector.scalar_tensor_tensor(
                        out=diff[:, 0:span], in0=e_b[:, 1, 0:span],
                        scalar=lam_p[:], in1=e_b[:, 0, 0:span],
                        op0=OP.mult, op1=OP.add)
                    if debug and pair == 0 and qb == 3:
                        de = pc2.tile([P, L], f32, tag="de")
                        nc.vector.tensor_copy(de[:, 0:span], e_b[:, 0, 0:span])
                        nc.sync.dma_start(dbg["d_e0"][:], de[:])
                        nc.vector.tensor_copy(de[:, 0:span], diff[:, 0:span])
                        nc.sync.dma_start(dbg["d_diff3"][:], de[:])
                    dT_bufs[(pair, qb)] = diff

                def emit_pv(pair, qb):
                    # PE transposes group g while DVE relu-copies g-1 and PE
                    # PVs g-1 (software-pipelined); trails the diff by an
                    # iteration so DVE's queue never stalls PE
                    diff = dT_bufs.pop((pair, qb))
                    u = 2 * qb + pair
                    span = (qb + 1) * P
                    pvh = pv_ctr[0] % 2
                    pv_ctr[0] += 1
                    pv = pv_buf(pvh)
                    nkb = qb + 1
                    grps = []

                    def rpv(g):
                        kb0, ng, trp = g
                        dT = pc2.tile([P, 512], bf16, tag="dT", bufs=3)
                        nc.vector.tensor_scalar(
                            out=dT[:, 0:ng * P], in0=trp[:, 0:ng * P],
                            scalar1=0.0, scalar2=None, op0=OP.max)
                        for i in range(ng):
                            kb = kb0 + i
                            nc.tensor.matmul(
                                pv[:], dT[:, i * P:(i + 1) * P],
                                vm[:, kb, :],
                                start=(kb == 0), stop=(kb == nkb - 1))

                    for grp in range((nkb + 3) // 4):
                        kb0 = grp * 4
                        ng = min(4, nkb - kb0)
                        trp = tr_half()
                        for i in range(ng):
                            nc.tensor.transpose(
                                trp[:, i * P:(i + 1) * P],
                                diff[:, (kb0 + i) * P:(kb0 + i + 1) * P],
                                ident_bf[:])
                        grps.append((kb0, ng, trp))
                        if len(grps) >= 2:
                            rpv(grps.pop(0))
                    while grps:
                        rpv(grps.pop(0))
                    # stash out1 (DVE), then ssq from the SBUF copy
                    o1 = out1_all[:, qb, pair * D:(pair + 1) * D]
                    nc.vector.tensor_copy(o1, pv[:])
                    nc.vector.scalar_tensor_tensor(
                        out=sq_scr[:], in0=o1, scalar=1.0, in1=o1,
                        op0=OP.mult, op1=OP.mult,
                        accum_out=ssqb[:, u:u + 1])

                def _rms_pair(qb0, nqb2):
                    # rsqrt via ln+exp for qbs [qb0, qb0+nqb2) (cols 2*qb0..)
                    u0 = 2 * qb0
                    w = 2 * nqb2
                    rsq = pc2.tile([P, 4], f32, tag="rsq")
                    nc.gpsimd.tensor_tensor(
                        rsq[:, 0:w], rbuf1[:, u0:u0 + w], rbuf1[:, u0:u0 + w],
                        OP.mult)
                    uarg = pc2.tile([P, 4], f32, tag="uarg")
                    nc.vector.scalar_tensor_tensor(
                        out=uarg[:, 0:w], in0=rsq[:, 0:w],
                        scalar=float(D) * 1e-6,
                        in1=ssqb[:, u0:u0 + w], op0=OP.mult, op1=OP.add)
                    lnu = pc2.tile([P, 4], f32, tag="lnu")
                    nc.scalar.activation(lnu[:, 0:w], uarg[:, 0:w], AF.Ln,
                                         scale=1.0 / D)
                    nc.scalar.activation(scl[:, u0:u0 + w], lnu[:, 0:w],
                                         AF.Exp, scale=-0.5)

                def emit_qbdone(qb):
                    # batched: odd qb handles {qb-1, qb}
                    if qb % 2 == 0:
                        return
                    _rms_pair(qb - 1, 2)
                    for qbx in (qb - 1, qb):
                        u0 = 2 * qbx
                        for pair in range(2):
                            sl = slice(pair * D, (pair + 1) * D)
                            nc.vector.tensor_scalar(
                                out=out1n[:, qbx, sl],
                                in0=out1_all[:, qbx, sl],
                                scalar1=scl[:, u0 + pair:u0 + pair + 1],
                                scalar2=None, op0=OP.mult)
                        if qbx < 14:
                            nc.sync.dma_start_transpose(
                                onT[:, qbx * P:(qbx + 1) * P],
                                out1n[:, qbx, :])
                        else:
                            trp = tr_half()
                            nc.tensor.transpose(
                                trp[:, 0:P], out1n[:, qbx, :], ident_bf[:])
                            nc.scalar.copy(onT[:, qbx * P:(qbx + 1) * P],
                                           trp[:, 0:P])
                        tgt = a2_inA if qbx < 8 else a2_inB
                        blk = qbx % 8
                        nc.sync.dma_start(
                            tgt[blk * P:(blk + 1) * P, :],
                            onT[:, qbx * P:(qbx + 1) * P])

                def emit_a2a(which):
                    a2i = a2_inA if which == 0 else a2_inB
                    a2o = a2_outA if which == 0 else a2_outB
                    if mock_collectives:
                        nc.sync.dma_start(a2o[:], a2i[:])
                    else:
                        nc.gpsimd.collective_compute(
                            "AllToAll", OP.bypass,
                            replica_groups=[list(range(N_CORES))],
                            ins=[a2i.opt()], outs=[a2o.opt()])

                # ---- the pipeline ----
                # per iteration lg: qkv(lg+1), T(lg-1), qbdone(lg-4),
                # rope(lg), fronts(lg-1), diffs(lg-2), pvs(lg-3)
                pv_done = set()
                qb_done = set()

                def maybe_pv(qb):
                    if 0 <= qb < NQB and qb not in pv_done:
                        pv_done.add(qb)
                        emit_pv(0, qb)
                        emit_pv(1, qb)

                def maybe_qbdone(qb):
                    if 0 <= qb < NQB and qb not in qb_done:
                        qb_done.add(qb)
                        emit_qbdone(qb)

                for lg in range(NQB):
                    if lg == 0:
                        emit_qkv01()
                    elif lg < NQB - 1:
                        emit_qkv(lg + 1)
                    if lg >= 1:
                        emit_T(lg - 1)
                    # early qbs finish shallow so A2A#1 fires at lg 10 and
                    # the Wo first half hoists into the ACT-paced mid-body;
                    # late qbs keep the deep trailing that hides DVE latency
                    if lg - 3 <= 7:
                        maybe_qbdone(lg - 3)
                    maybe_qbdone(lg - 6)
                    if lg == 10:
                        emit_a2a(0)
                        nc.scalar.dma_start(
                            omTA[:],
                            a2_outA[:].rearrange("(c p) l -> p c l", p=P))
                    emit_rope(lg)
                    if lg >= 1:
                        emit_T_copies(lg - 1)
                    if lg >= 1:
                        emit_front(0, lg - 1)
                    if lg >= 2:
                        emit_back(0, lg - 2)
                    if lg >= 1:
                        emit_front(1, lg - 1)
                    if lg >= 2:
                        emit_back(1, lg - 2)
                    if lg - 2 <= 7:
                        maybe_pv(lg - 2)
                    maybe_pv(lg - 5)
                psQ_cm.__exit__(None, None, None)

                emit_T(NQB - 1)
                emit_T_copies(NQB - 1)
                emit_front(0, NQB - 1)
                emit_back(0, NQB - 2)
                maybe_pv(NQB - 5)
                emit_front(1, NQB - 1)
                emit_back(1, NQB - 2)
                maybe_qbdone(NQB - 6)
                emit_back(0, NQB - 1)
                maybe_pv(NQB - 4)
                emit_back(1, NQB - 1)
                maybe_qbdone(NQB - 5)
                maybe_pv(NQB - 3)
                maybe_qbdone(NQB - 4)
                maybe_pv(NQB - 2)
                maybe_qbdone(NQB - 3)
                maybe_pv(NQB - 1)
                maybe_qbdone(NQB - 2)
                maybe_qbdone(NQB - 1)
                emit_a2a(1)
                omTB_r = a2_outB[:].rearrange("(c p) l -> p c l", p=P)
                nc.scalar.dma_start(omTB[:, 0:4, :], omTB_r[:, 0:4, :])
                nc.scalar.dma_start(omTB[:, 4:8, :], omTB_r[:, 4:8, :])

                # ---------------- Wo (A overlaps the second A2A) ----------
                with tc.tile_pool(name="psD", bufs=2, space="PSUM") as psD:
                    def emit_wo(half, omT):
                        # last column group split finer so the final
                        # copy+DMA teardown chain is short
                        groups = [(0, 512), (512, 512), (1024, 512),
                                  (1536, 384), (1920, 128)]
                        for g0, gw in groups:
                            csl = slice(g0, g0 + gw)
                            ops = psD.tile([P, 512], f32, tag="ops", bufs=2)
                            for dchunk in range(N_CORES):
                                nc.tensor.matmul(
                                    ops[:, 0:gw], omT[:, dchunk, :],
                                    wo_sb[:, dchunk, csl],
                                    start=(dchunk == 0),
                                    stop=(dchunk == N_CORES - 1))
                            o_sb = pc2.tile([P, 512], bf16, tag="o_sb",
                                            bufs=2)
                            nc.scalar.copy(o_sb[:, 0:gw], ops[:, 0:gw])
                            nc.sync.dma_start(
                                out_d[half * P:(half + 1) * P, csl],
                                o_sb[:, 0:gw])

                    emit_wo(0, omTA)
                    emit_wo(1, omTB)

                if debug:
                    dqt = pp.tile([P, L], f32, tag="dqt")
                    nc.vector.tensor_copy(dqt[:], qTs[0][:])
                    nc.sync.dma_start(dbg["d_qT0"][:], dqt[:])
                    nc.vector.tensor_copy(dqt[:], kT[:])
                    nc.sync.dma_start(dbg["d_kT"][:], dqt[:])
                    dvm = pp.tile([P, D], f32, tag="dvm")
                    nc.vector.tensor_copy(dvm[:], vm[:, 3, :])
                    nc.sync.dma_start(dbg["d_vm"][:], dvm[:])
                    nc.sync.dma_start(dbg["d_r1"][:], rbuf1[:])
                    nc.sync.dma_start(dbg["d_r2"][:], rbuf2[:])
                    nc.sync.dma_start(dbg["d_ssq"][:], ssqb[:])
                    nc.sync.dma_start(dbg["d_scl"][:], scl[:])
                    nc.sync.dma_start(dbg["d_out1"][:], out1_all[:])
                    nc.vector.tensor_copy(dqt[:], onT[:])
                    nc.sync.dma_start(dbg["d_onT"][:], dqt[:])

    return nc


_CACHE = {}


def _get_program():
    if "nc" not in _CACHE:
        nc = _build()
        nc.compile()
        _CACHE["nc"] = nc
    return _CACHE["nc"]


def _host_lambda(x2, cos, sin, Wq, Wk, lq1, lk1, lq2, lk2):
    """Exact lambda: the dots are linear+rope functionals of x."""
    c32 = cos[:L, :32]
    s32 = sin[:L, :32]

    def rt(lam):
        l1, l2 = lam[:32], lam[32:]
        return np.concatenate([c32 * l1 + s32 * l2, -s32 * l1 + c32 * l2],
                              axis=1)  # [L, 64] rows R_l^T lam

    Wq3 = Wq.reshape(HID, H, D)
    Wqe = Wq3[:, 0::2, :].sum(axis=1)
    Wqo = Wq3[:, 1::2, :].sum(axis=1)
    Wks = Wk.reshape(HID, HKV, D).sum(axis=1)
    pk = x2 @ Wks
    d1 = np.clip(((x2 @ Wqe) * rt(lq1)).sum() / L, -10.0, 10.0)
    d2 = np.clip(2.0 * (pk * rt(lk1)).sum() / L, -10.0, 10.0)
    d3 = np.clip(((x2 @ Wqo) * rt(lq2)).sum() / L, -10.0, 10.0)
    d4 = np.clip(2.0 * (pk * rt(lk2)).sum() / L, -10.0, 10.0)
    lam = np.exp(d1) * np.exp(d2) - np.exp(d3) * np.exp(d4) + LAMBDA_INIT
    return float(np.clip(lam, 0.0, 1.0))


def _host_prep(x, cos, sin, Wq, Wk, Wv, Wo, lambda_q1, lambda_k1, lambda_q2,
               lambda_k2, subln_weight):
    bf = ml_dtypes.bfloat16
    x2 = np.asarray(x, np.float32).reshape(L, HID)
    cos = np.asarray(cos, np.float32)
    sin = np.asarray(sin, np.float32)
    # xt swizzled for 512B-contiguous lg-major streaming:
    # xtp[lg, ktp, p, e*128+col] = x2[lg*128+col, (2*ktp+e)*128+p]
    xT = np.ascontiguousarray(x2.T)                      # [HID, L]
    Bv = xT.reshape(8, 2, P, NQB, P)                     # [ktp, e, p, lg, col]
    xtp = np.ascontiguousarray(
        Bv.transpose(3, 0, 2, 1, 4).reshape(NQB * 8 * P, 256)).astype(bf)
    ropet = np.ascontiguousarray(
        np.concatenate([cos[:L, :32], sin[:L, :32]], axis=1))  # [L, 64]
    Wq = np.asarray(Wq, np.float32)
    Wk = np.asarray(Wk, np.float32)
    Wv = np.asarray(Wv, np.float32)
    lq1 = np.asarray(lambda_q1, np.float32)
    lq2 = np.asarray(lambda_q2, np.float32)
    lk1 = np.asarray(lambda_k1, np.float32)
    lk2 = np.asarray(lambda_k2, np.float32)
    lam = _host_lambda(x2, cos, sin, Wq, Wk, lq1, lk1, lq2, lk2)
    lamneg = np.full((1, 1), -lam, np.float32)
    s = np.asarray(subln_weight, np.float32) * (1.0 - LAMBDA_INIT)   # [128]
    Wo = np.asarray(Wo, np.float32)
    wo_eff = np.empty((H * D // 2, HID), np.float32)
    for p in range(H // 2):
        blk = Wo[p * 2 * D:(p + 1) * 2 * D, :]           # [128, HID]
        wo_eff[p * D:(p + 1) * D] = (s[:D, None] * blk[:D]
                                     + s[D:, None] * blk[D:])
    wo_eff = wo_eff.astype(bf)
    in_maps = []
    for c in range(N_CORES):
        wqkv_c = np.concatenate([
            Wq[:, c * NH * D:(c + 1) * NH * D],
            Wk[:, c * D:(c + 1) * D],
            Wv[:, c * D:(c + 1) * D]], axis=1).astype(bf)    # [HID, 384]
        in_maps.append({
            "xtp": xtp, "wqkv": wqkv_c, "ropet": ropet, "lamneg": lamneg,
            "wo": wo_eff,
        })
    return in_maps


def kernel(**inputs) -> np.ndarray:
    nc = _get_program()
    in_maps = _host_prep(**{k: v for k, v in inputs.items() if k != "mask"})
    res = run_bass_kernel_spmd(nc, in_maps, list(range(N_CORES)))
    out = np.empty((L, HID), np.float32)
    for c in range(N_CORES):
        r = np.asarray(res.results[c]["out"], np.float32)
        out[c * P:(c + 1) * P] = r[0:P]
        out[(c + 8) * P:(c + 9) * P] = r[P:2 * P]
    return out.reshape(1, L, HID)


# revision 6
# speedup vs baseline: 1.0558x; 1.0141x over previous
"""Trainium2 Bass kernel for nn_DifferentialGQA (8-core SPMD), v3.

Fully-fused single-phase pipeline (tensor-parallel, no mid-kernel AllReduce):
  - lambda is computed EXACTLY on the host (it is a linear+rope functional of
    x and the weights), so the AllReduce and the phase-A/phase-C serialization
    of v2 disappear entirely.
  - Per 128-row block lg: QKV matmuls (PE) -> rope (DVE stage + GPSIMD) ->
    q/k transposes (PE) -> attention fronts for qb=lg (score matmuls +
    diag-mask matmul; ACT exp with free row-sum accumulation) -> backs
    trailing one lg (fused diff stt on DVE, PE transposes + PV) -> per-qb RMS
    + output transpose trailing three lgs. ACT's exp stream overlaps the QKV
    GEMMs; PE stays continuously busy (p-state friendly).
  - Engine budget: PE ~117us (the roofline), ACT ~97us (exp+accum), DVE ~85us
    (copies + fused diff), Pool ~65us (ropes + small elementwise).
  - x^T is host-swizzled so each lg-major stream chunk is 512B-contiguous
    (full DMA rate); PE starts after ~6us.
  - Output reshard uses TWO AllToAlls over an interleaved qb->core map
    (core c owns L rows of q-blocks c and c+8): the first fires mid-kernel
    (fully hidden), the second at the end overlaps the first Wo half.
  - RMS rsqrt via ln+exp (both live in the natural_log_exp activation table:
    zero table swaps).
"""
import sys

sys.path.insert(0, "/opt/trn_rl_repo")

import numpy as np
import ml_dtypes

import concourse.bass as bass
import concourse.mybir as mybir
import concourse.tile as tile
from concourse import bacc
from concourse.bass_utils import run_bass_kernel_spmd
from concourse.hw_specs import get_activation_tables
from concourse.masks import make_identity

dt = mybir.dt
AF = mybir.ActivationFunctionType
OP = mybir.AluOpType

N_CORES = 8
L = 2048
HID = 2048
H = 32
HKV = 8
D = 64
CAP = 50.0
LAMBDA_INIT = 0.8 - 0.6 * float(np.exp(-0.3 * 4))
P = 128
LROWS = L // N_CORES          # 256 output rows per core
NQB = L // P                  # 16 query blocks
KT = HID // P                 # 16 contraction tiles
NH = H // N_CORES             # 4 q heads per core
SCALE = 1.0 / float(np.sqrt(D))
SCHUNK = 1024                 # exp chunk (2 PSUM banks of f32)


def _build(mock_collectives: bool = False, debug: bool = False):
    nc = bacc.Bacc("TRN2", target_bir_lowering=False, debug=False,
                   num_devices=(1 if mock_collectives else N_CORES))
    f32, bf16 = dt.float32, dt.bfloat16

    xtp = nc.dram_tensor("xtp", [NQB * 8 * P, 256], bf16,
                         kind="ExternalInput").ap()
    wqkv = nc.dram_tensor("wqkv", [HID, 384], bf16, kind="ExternalInput").ap()
    ropet = nc.dram_tensor("ropet", [L, 64], f32, kind="ExternalInput").ap()
    lam_in = nc.dram_tensor("lamneg", [1, 1], f32, kind="ExternalInput").ap()
    wo = nc.dram_tensor("wo", [H * D // 2, HID], bf16, kind="ExternalInput").ap()
    out_d = nc.dram_tensor("out", [LROWS, HID], bf16, kind="ExternalOutput").ap()
    dbg = {}
    if debug:
        for nm, shp, dty in [
            ("d_q0", [P, 256], f32), ("d_k0", [P, D], f32),
            ("d_qT0", [P, L], f32), ("d_kT", [P, L], f32),
            ("d_vm", [P, D], f32), ("d_lam", [P, 1], f32),
            ("d_r1", [P, 32], f32), ("d_r2", [P, 32], f32),
            ("d_e0", [P, L], f32), ("d_diff3", [P, L], f32),
            ("d_out1", [P, NQB, P], f32), ("d_ssq", [P, 32], f32),
            ("d_scl", [P, 32], f32), ("d_onT", [P, L], f32),
        ]:
            dbg[nm] = nc.dram_tensor(nm, shp, dty, kind="ExternalOutput").ap()

    with tile.TileContext(nc) as tc:
        with (
            tc.tile_pool(name="persist", bufs=1) as pp,
            tc.tile_pool(name="dram", bufs=1, space="DRAM") as dram,
        ):
            a2_inA = dram.tile([N_CORES * P, P], bf16, tag="a2_inA")
            a2_outA = dram.tile([N_CORES * P, P], bf16, tag="a2_outA")
            a2_inB = dram.tile([N_CORES * P, P], bf16, tag="a2_inB")
            a2_outB = dram.tile([N_CORES * P, P], bf16, tag="a2_outB")

            # pin the act table that covers BOTH Exp and Ln so the table-load
            # pass never swaps (it honors pre-placed loads)
            _tabs = list(get_activation_tables(nc.m.arch).keys())
            nc.scalar.add_instruction(mybir.InstLoadActFuncSet(
                name=nc.get_next_instruction_name(), ins=[], outs=[],
                act_func_set_id=_tabs.index("natural_log_exp_and_others")))

            ident_bf = pp.tile([P, P], bf16, tag="ident_bf")
            make_identity(nc, ident_bf[:])
            # additive causal mask for the diagonal block: 0 on/below diag,
            # -1e9 above; applied by PE as an accumulating matmul with the
            # identity as stationary (GPSIMD cannot touch PSUM)
            cmask = pp.tile([P, P], bf16, tag="cmask")
            nc.gpsimd.memset(cmask[:], 0.0)
            nc.gpsimd.affine_select(
                out=cmask[:], in_=cmask[:], compare_op=OP.is_ge, fill=-1e9,
                base=0, pattern=[[-1, P]], channel_multiplier=1)

            # persistent tensors
            qkT_all = pp.tile([P, 3, L], bf16, tag="qkT")
            qTs = [qkT_all[:, 0, :], qkT_all[:, 1, :]]
            kT = qkT_all[:, 2, :]       # kv head on both halves
            vm = pp.tile([P, NQB, D], bf16, tag="vm")  # v rows [l, d]
            lamneg_bc = pp.tile([P, 1], f32, tag="lamneg")
            # col u = 2*qb + pair
            rbuf1 = pp.tile([P, 32], f32, tag="rbuf1")
            rbuf2 = pp.tile([P, 32], f32, tag="rbuf2")
            ssqb = pp.tile([P, 32], f32, tag="ssqb")
            scl = pp.tile([P, 32], f32, tag="scl")
            out1_all = pp.tile([P, NQB, P], f32, tag="out1")  # [q, qb, 2x64]
            out1n = pp.tile([P, NQB, P], bf16, tag="out1n")
            onT = pp.tile([P, L], bf16, tag="onT")            # [dcat, L]

            xt_sb = pp.tile([P, NQB, KT, P], bf16, tag="xt")  # lg-major
            wqkv_sb = pp.tile([P, KT, 384], bf16, tag="wqkv")
            rope_sb = pp.tile([P, NQB, 64], f32, tag="rope")
            wo_sb = pp.tile([P, N_CORES, HID], bf16, tag="wo_sb")
            lam_sb = pp.tile([1, 1], f32, tag="lam_sb")
            sq_scr = pp.tile([P, D], f32, tag="sq_scr")
            omTA = pp.tile([P, N_CORES, P], bf16, tag="omTA")
            omTB = pp.tile([P, N_CORES, P], bf16, tag="omTB")

            # ---------------- DMA prologue ----------------
            # xt chunk 0 + the first wqkv quarter gate the first QKV matmul:
            # they go first; everything else trails.
            def emit_xt_dma(lg, eng):
                src = xtp[lg * 8 * P:(lg + 1) * 8 * P, :].rearrange(
                    "(k p) c -> p k c", p=P)
                dst = xt_sb[:, lg, :, :].rearrange("p k c -> p (k c)")
                dst = dst.rearrange("p (k c) -> p k c", k=8)
                eng.dma_start(dst, src)

            wqkv_r = wqkv[:].rearrange("(t p) c -> p t c", p=P)
            nc.scalar.dma_start(wqkv_sb[:, 0:4, :], wqkv_r[:, 0:4, :])

            def emit_xt_half(lg, half, eng):
                k0 = half * 4
                src_ = xtp[lg * 8 * P + k0 * P:lg * 8 * P + (k0 + 4) * P,
                           :].rearrange("(k p) c -> p k c", p=P)
                dst = xt_sb[:, lg, 2 * k0:2 * (k0 + 4), :].rearrange(
                    "p k c -> p (k c)").rearrange("p (k c) -> p k c", k=4)
                eng.dma_start(dst, src_)

            emit_xt_half(0, 0, nc.sync)
            emit_xt_half(0, 1, nc.sync)
            nc.scalar.dma_start(wqkv_sb[:, 4:8, :], wqkv_r[:, 4:8, :])
            emit_xt_half(1, 0, nc.sync)
            emit_xt_half(1, 1, nc.sync)
            nc.scalar.dma_start(wqkv_sb[:, 8:12, :], wqkv_r[:, 8:12, :])
            emit_xt_dma(2, nc.sync)
            nc.scalar.dma_start(wqkv_sb[:, 12:16, :], wqkv_r[:, 12:16, :])
            emit_xt_dma(3, nc.sync)
            nc.scalar.dma_start(lam_sb[:], lam_in[:])
            nc.sync.dma_start(
                rope_sb[:], ropet[:].rearrange("(g p) c -> p g c", p=P))
            for lg in range(4, NQB):
                emit_xt_dma(lg, nc.sync if lg % 2 == 0 else nc.scalar)
            wo_r = wo[:].rearrange("(c p) n -> p c n", p=P)
            nc.scalar.dma_start(wo_sb[:, 0:4, :], wo_r[:, 0:4, :])
            nc.scalar.dma_start(wo_sb[:, 4:8, :], wo_r[:, 4:8, :])

            nc.gpsimd.partition_broadcast(lamneg_bc[:], lam_sb[:])
            if debug:
                nc.sync.dma_start(dbg["d_lam"][:], lamneg_bc[:])

            # ---------------- fused main pipeline ----------------
            with (
                tc.tile_pool(name="pa2", bufs=2) as pa2,
                tc.tile_pool(name="pc2", bufs=2) as pc2,
                tc.tile_pool(name="psS", bufs=2, space="PSUM") as psS,
                tc.tile_pool(name="psT", bufs=1, space="PSUM") as psT,
            ):
                psQ_cm = tc.tile_pool(name="psQ", bufs=2, space="PSUM")
                psQ = psQ_cm.__enter__()

                pvx = psT.tile([P, 512], f32, tag="pvx")
                trn_t = psT.tile([P, 1024], bf16, tag="tr")
                tr_ctr = [0]

                def tr_half():
                    h = tr_ctr[0] % 2
                    tr_ctr[0] += 1
                    return trn_t[:, h * 512:(h + 1) * 512]

                def pv_buf(i):
                    return pvx[:, i * P:i * P + D]

                qkv_ps = {}

                def emit_qkv(lg):
                    ps = psQ.tile([P, 384], f32, tag="qkv", bufs=2,
                                  name=f"qkv{lg}")[:]
                    qkv_ps[lg] = ps
                    for kt in range(KT):
                        nc.tensor.matmul(
                            ps, xt_sb[:, lg, kt, :],
                            wqkv_sb[:, kt, :],
                            start=(kt == 0), stop=(kt == KT - 1))

                def emit_qkv01():
                    # kt-outer over the first two lgs so PE consumes the
                    # streaming wqkv quarters at half rate (no head stall)
                    qkv_ps[0] = psQ.tile([P, 384], f32, tag="qkv",
                                         bufs=2, name="qkv0")[:]
                    # lg1 borrows the (not-yet-used) pv bank as its
                    # accumulator during the kt-outer warmup
                    qkv_ps[1] = pvx[:, 0:384]
                    for kt in range(KT):
                        for lg in (0, 1):
                            nc.tensor.matmul(
                                qkv_ps[lg], xt_sb[:, lg, kt, :],
                                wqkv_sb[:, kt, :],
                                start=(kt == 0), stop=(kt == KT - 1))

                ropes = {}

                def emit_rope(lg):
                    # DVE stages qkv PSUM into SBUF; GPSIMD (Pool) does the
                    # rope math and the v copy, keeping ACT exp-only. The
                    # first two ropes go straight through DVE from PSUM (DVE
                    # is idle at the head and the stage+Pool chain would
                    # delay T(0)/T(1) by ~1.5us each).
                    ps = qkv_ps.pop(lg)
                    if lg < 3:
                        nc.vector.tensor_copy(vm[:, lg, :], ps[:, 320:384])
                        qeng = nc.vector
                        qsrc = ps
                    else:
                        stage = pa2.tile([P, 384], f32, tag="stg", bufs=4)
                        nc.vector.tensor_copy(stage[:], ps)
                        nc.gpsimd.tensor_copy(vm[:, lg, :], stage[:, 320:384])
                        qeng = nc.gpsimd
                        qsrc = stage
                    # ---- rope q: [128, 4h, 64] ----
                    qk_sb = pa2.tile([P, 384], bf16, tag="qk_sb", bufs=4)
                    q_sb = qk_sb[:, 0:256]
                    ta = pa2.tile([P, 4, 32], f32, tag="ta")
                    tb = pa2.tile([P, 4, 32], f32, tag="tb")
                    qp3 = qsrc[:, 0:256].rearrange("p (h j) -> p h j", j=D)
                    q3 = q_sb.rearrange("p (h j) -> p h j", j=D)
                    c3 = rope_sb[:, lg, 0:32].unsqueeze(1).broadcast_to([P, 4, 32])
                    s3 = rope_sb[:, lg, 32:64].unsqueeze(1).broadcast_to([P, 4, 32])
                    qeng.tensor_tensor(ta[:], qp3[:, :, 32:64], s3, OP.mult)
                    qeng.tensor_tensor(tb[:], qp3[:, :, 0:32], s3, OP.mult)
                    qeng.tensor_tensor(q3[:, :, 0:32], qp3[:, :, 0:32], c3, OP.mult)
                    qeng.tensor_tensor(q3[:, :, 32:64], qp3[:, :, 32:64], c3, OP.mult)
                    qeng.tensor_tensor(q3[:, :, 0:32], q3[:, :, 0:32], ta[:], OP.subtract)
                    qeng.tensor_tensor(q3[:, :, 32:64], q3[:, :, 32:64], tb[:], OP.add)
                    # ---- rope k: [128, 64] ----
                    k_sb = qk_sb[:, 256:320]
                    kc = rope_sb[:, lg, 0:32]
                    ks = rope_sb[:, lg, 32:64]
                    kta = pa2.tile([P, 32], f32, tag="kta")
                    ktb = pa2.tile([P, 32], f32, tag="ktb")
                    qeng.tensor_tensor(kta[:], qsrc[:, 288:320], ks, OP.mult)
                    qeng.tensor_tensor(ktb[:], qsrc[:, 256:288], ks, OP.mult)
                    qeng.tensor_tensor(k_sb[:, 0:32], qsrc[:, 256:288], kc, OP.mult)
                    qeng.tensor_tensor(k_sb[:, 32:64], qsrc[:, 288:320], kc, OP.mult)
                    qeng.tensor_tensor(k_sb[:, 0:32], k_sb[:, 0:32], kta[:], OP.subtract)
                    qeng.tensor_tensor(k_sb[:, 32:64], k_sb[:, 32:64], ktb[:], OP.add)
                    # duplicate k into the second half so the fused XBAR
                    # transpose lands [kT | kT] on the partition halves
                    qeng.tensor_copy(qk_sb[:, 320:384], k_sb)
                    ropes[lg] = qk_sb
                    if debug and lg == 0:
                        dq0 = pp.tile([P, 256], f32, tag="dq0")
                        nc.vector.tensor_copy(dq0[:], q_sb[:])
                        nc.sync.dma_start(dbg["d_q0"][:], dq0[:])
                        dk0 = pp.tile([P, D], f32, tag="dk0")
                        nc.vector.tensor_copy(dk0[:], k_sb)
                        nc.sync.dma_start(dbg["d_k0"][:], dk0[:])

                T_pend = {}

                def emit_T(lg):
                    qk_sb = ropes.pop(lg)
                    trp = tr_half()
                    for b in range(3):
                        nc.tensor.transpose(
                            trp[:, b * P:(b + 1) * P],
                            qk_sb[:, b * P:(b + 1) * P], ident_bf[:])
                    T_pend[lg] = trp

                def emit_T_copies(lg):
                    # deferred to the END of the iteration's DVE queue so
                    # DVE's dep-free work (diff/relu) runs first
                    trp = T_pend.pop(lg)
                    for b in range(3):
                        nc.vector.tensor_copy(
                            qkT_all[:, b, lg * P:(lg + 1) * P],
                            trp[:, b * P:(b + 1) * P])

                e_bufs = {}
                dT_bufs = {}
                pv_ctr = [0]

                def emit_front(pair, qb):
                    # scores + mask + exp (+rowsums) for one unit
                    qTp = qTs[pair]
                    u = 2 * qb + pair
                    span = (qb + 1) * P
                    e_b = pc2.tile([P, 2, L], bf16, tag="e", bufs=3)
                    for t in range(2):
                        rdst = rbuf1 if t == 0 else rbuf2
                        nchunk = (span + SCHUNK - 1) // SCHUNK
                        # small remainder chunk FIRST so the unit's first exp
                        # fires after less PE work (earlier ACT start)
                        bounds = [0, span - SCHUNK] if nchunk == 2 else [0]
                        for ch, c0 in enumerate(bounds):
                            csp = (span - SCHUNK if nchunk == 2 and ch == 0
                                   else min(SCHUNK, span - c0))
                            sps = psS.tile([P, SCHUNK], f32, tag="scores")
                            for m0 in range(0, csp, 512):
                                msp = min(512, csp - m0)
                                nc.tensor.matmul(
                                    sps[:, m0:m0 + msp],
                                    qTp[t * D:(t + 1) * D, qb * P:(qb + 1) * P],
                                    kT[t * D:(t + 1) * D, c0 + m0:c0 + m0 + msp],
                                    start=True, stop=True)
                            dlo = qb * P - c0
                            if 0 <= dlo < csp:
                                # causal mask: PE accumulates -1e9 above the
                                # diagonal (identity-stationary add)
                                nc.tensor.matmul(
                                    sps[:, dlo:dlo + P], ident_bf[:],
                                    cmask[:], start=False, stop=True,
                                    skip_group_check=True)
                            if ch == 0:
                                acc_ap = rdst[:, u:u + 1]
                            else:
                                rtmp = pc2.tile([P, 1], f32, tag="rtmp",
                                                bufs=4)
                                acc_ap = rtmp[:]
                            nc.scalar.activation(
                                e_b[:, t, c0:c0 + csp], sps[:, 0:csp],
                                AF.Exp, scale=SCALE, accum_out=acc_ap)
                            if ch > 0:
                                nc.vector.tensor_tensor(
                                    rdst[:, u:u + 1], rdst[:, u:u + 1],
                                    rtmp[:], OP.add)
                    e_bufs[(pair, qb)] = e_b

                def emit_back(pair, qb):
                    # fused diff + transpose + PV for one unit
                    e_b = e_bufs.pop((pair, qb))
                    u = 2 * qb + pair
                    span = (qb + 1) * P
                    # lam_pr = -lam * r1 / r2, then diff = e1 + lam_pr*e2 in
                    # a single scalar_tensor_tensor
                    lam_p = pc2.tile([P, 1], f32, tag="lam_p")
                    nc.vector.reciprocal(lam_p[:], rbuf2[:, u:u + 1])
                    nc.vector.tensor_tensor(
                        lam_p[:], lam_p[:], lamneg_bc[:], OP.mult)
                    nc.vector.tensor_tensor(
                        lam_p[:], lam_p[:], rbuf1[:, u:u + 1], OP.mult)
                    diff = pc2.tile([P, L], bf16, tag="diff", bufs=4)
                    nc.vector.scalar_tensor_tensor(
                        out=diff[:, 0:span], in0=e_b[:, 1, 0:span],
                        scalar=lam_p[:], in1=e_b[:, 0, 0:span],
                        op0=OP.mult, op1=OP.add)
                    if debug and pair == 0 and qb == 3:
                        de = pc2.tile([P, L], f32, tag="de")
                        nc.vector.tensor_copy(de[:, 0:span], e_b[:, 0, 0:span])
                        nc.sync.dma_start(dbg["d_e0"][:], de[:])
                        nc.vector.tensor_copy(de[:, 0:span], diff[:, 0:span])
                        nc.sync.dma_start(dbg["d_diff3"][:], de[:])
                    dT_bufs[(pair, qb)] = diff

                def emit_pv(pair, qb):
                    # PE transposes group g while DVE relu-copies g-1 and PE
                    # PVs g-1 (software-pipelined); trails the diff by an
                    # iteration so DVE's queue never stalls PE
                    diff = dT_bufs.pop((pair, qb))
                    u = 2 * qb + pair
                    span = (qb + 1) * P
                    pvh = pv_ctr[0] % 2
                    pv_ctr[0] += 1
                    pv = pv_buf(pvh)
                    nkb = qb + 1
                    grps = []

                    def rpv(g):
                        kb0, ng, trp = g
                        dT = pc2.tile([P, 512], bf16, tag="dT", bufs=3)
                        nc.vector.tensor_scalar(
                            out=dT[:, 0:ng * P], in0=trp[:, 0:ng * P],
                            scalar1=0.0, scalar2=None, op0=OP.max)
                        for i in range(ng):
                            kb = kb0 + i
                            nc.tensor.matmul(
                                pv[:], dT[:, i * P:(i + 1) * P],
                                vm[:, kb, :],
                                start=(kb == 0), stop=(kb == nkb - 1))

                    for grp in range((nkb + 3) // 4):
                        kb0 = grp * 4
                        ng = min(4, nkb - kb0)
                        trp = tr_half()
                        for i in range(ng):
                            nc.tensor.transpose(
                                trp[:, i * P:(i + 1) * P],
                                diff[:, (kb0 + i) * P:(kb0 + i + 1) * P],
                                ident_bf[:])
                        grps.append((kb0, ng, trp))
                        if len(grps) >= 2:
                            rpv(grps.pop(0))
                    while grps:
                        rpv(grps.pop(0))
                    # stash out1 (DVE), then ssq from the SBUF copy
                    o1 = out1_all[:, qb, pair * D:(pair + 1) * D]
                    nc.vector.tensor_copy(o1, pv[:])
                    nc.vector.scalar_tensor_tensor(
                        out=sq_scr[:], in0=o1, scalar=1.0, in1=o1,
                        op0=OP.mult, op1=OP.mult,
                        accum_out=ssqb[:, u:u + 1])

                def _rms_pair(qb0, nqb2):
                    # rsqrt via ln+exp for qbs [qb0, qb0+nqb2) (cols 2*qb0..)
                    u0 = 2 * qb0
                    w = 2 * nqb2
                    rsq = pc2.tile([P, 4], f32, tag="rsq")
                    nc.gpsimd.tensor_tensor(
                        rsq[:, 0:w], rbuf1[:, u0:u0 + w], rbuf1[:, u0:u0 + w],
                        OP.mult)
                    uarg = pc2.tile([P, 4], f32, tag="uarg")
                    nc.vector.scalar_tensor_tensor(
                        out=uarg[:, 0:w], in0=rsq[:, 0:w],
                        scalar=float(D) * 1e-6,
                        in1=ssqb[:, u0:u0 + w], op0=OP.mult, op1=OP.add)
                    lnu = pc2.tile([P, 4], f32, tag="lnu")
                    nc.scalar.activation(lnu[:, 0:w], uarg[:, 0:w], AF.Ln,
                                         scale=1.0 / D)
                    nc.scalar.activation(scl[:, u0:u0 + w], lnu[:, 0:w],
                                         AF.Exp, scale=-0.5)

                def emit_qbdone(qb):
                    # batched: odd qb handles {qb-1, qb}
                    if qb % 2 == 0:
                        return
                    _rms_pair(qb - 1, 2)
                    for qbx in (qb - 1, qb):
                        u0 = 2 * qbx
                        for pair in range(2):
                            sl = slice(pair * D, (pair + 1) * D)
                            nc.vector.tensor_scalar(
                                out=out1n[:, qbx, sl],
                                in0=out1_all[:, qbx, sl],
                                scalar1=scl[:, u0 + pair:u0 + pair + 1],
                                scalar2=None, op0=OP.mult)
                        if qbx < 14:
                            nc.sync.dma_start_transpose(
                                onT[:, qbx * P:(qbx + 1) * P],
                                out1n[:, qbx, :])
                        else:
                            trp = tr_half()
                            nc.tensor.transpose(
                                trp[:, 0:P], out1n[:, qbx, :], ident_bf[:])
                            nc.scalar.copy(onT[:, qbx * P:(qbx + 1) * P],
                                           trp[:, 0:P])
                        tgt = a2_inA if qbx < 8 else a2_inB
                        blk = qbx % 8
                        nc.sync.dma_start(
                            tgt[blk * P:(blk + 1) * P, :],
                            onT[:, qbx * P:(qbx + 1) * P])

                def emit_a2a(which):
                    a2i = a2_inA if which == 0 else a2_inB
                    a2o = a2_outA if which == 0 else a2_outB
                    if mock_collectives:
                        nc.sync.dma_start(a2o[:], a2i[:])
                    else:
                        nc.gpsimd.collective_compute(
                            "AllToAll", OP.bypass,
                            replica_groups=[list(range(N_CORES))],
                            ins=[a2i.opt()], outs=[a2o.opt()])

                # ---- the pipeline ----
                # per iteration lg: qkv(lg+1), T(lg-1), qbdone(lg-4),
                # rope(lg), fronts(lg-1), diffs(lg-2), pvs(lg-3)
                pv_done = set()
                qb_done = set()

                def maybe_pv(qb):
                    if 0 <= qb < NQB and qb not in pv_done:
                        pv_done.add(qb)
                        emit_pv(0, qb)
                        emit_pv(1, qb)

                def maybe_qbdone(qb):
                    if 0 <= qb < NQB and qb not in qb_done:
                        qb_done.add(qb)
                        emit_qbdone(qb)

                for lg in range(NQB):
                    if lg == 0:
                        emit_qkv01()
                    elif lg < NQB - 1:
                        emit_qkv(lg + 1)
                    if lg >= 1:
                        emit_T(lg - 1)
                    # early qbs finish shallow so A2A#1 fires at lg 10 and
                    # the Wo first half hoists into the ACT-paced mid-body;
                    # late qbs keep the deep trailing that hides DVE latency
                    if lg - 3 <= 7:
                        maybe_qbdone(lg - 3)
                    maybe_qbdone(lg - 6)
                    if lg == 10:
                        emit_a2a(0)
                        nc.scalar.dma_start(
                            omTA[:],
                            a2_outA[:].rearrange("(c p) l -> p c l", p=P))
                    emit_rope(lg)
                    if lg >= 1:
                        emit_T_copies(lg - 1)
                    if lg >= 1:
                        emit_front(0, lg - 1)
                    if lg >= 2:
                        emit_back(0, lg - 2)
                    if lg >= 1:
                        emit_front(1, lg - 1)
                    if lg >= 2:
                        emit_back(1, lg - 2)
                    if lg - 2 <= 7:
                        maybe_pv(lg - 2)
                    maybe_pv(lg - 5)
                psQ_cm.__exit__(None, None, None)

                emit_T(NQB - 1)
                emit_T_copies(NQB - 1)
                emit_front(0, NQB - 1)
                emit_back(0, NQB - 2)
                maybe_pv(NQB - 5)
                emit_front(1, NQB - 1)
                emit_back(1, NQB - 2)
                maybe_qbdone(NQB - 6)
                emit_back(0, NQB - 1)
                maybe_pv(NQB - 4)
                emit_back(1, NQB - 1)
                maybe_qbdone(NQB - 5)
                maybe_pv(NQB - 3)
                maybe_qbdone(NQB - 4)
                maybe_pv(NQB - 2)
                maybe_qbdone(NQB - 3)
                maybe_pv(NQB - 1)
                maybe_qbdone(NQB - 2)
                maybe_qbdone(NQB - 1)
                emit_a2a(1)
                omTB_r = a2_outB[:].rearrange("(c p) l -> p c l", p=P)
                nc.scalar.dma_start(omTB[:, 0:4, :], omTB_r[:, 0:4, :])
                nc.scalar.dma_start(omTB[:, 4:8, :], omTB_r[:, 4:8, :])

                # ---------------- Wo (A overlaps the second A2A) ----------
                with tc.tile_pool(name="psD", bufs=2, space="PSUM") as psD:
                    def emit_wo(half, omT):
                        # last column group split finer so the final
                        # copy+DMA teardown chain is short
                        groups = [(0, 512), (512, 512), (1024, 512),
                                  (1536, 384), (1920, 128)]
                        for g0, gw in groups:
                            csl = slice(g0, g0 + gw)
                            ops = psD.tile([P, 512], f32, tag="ops", bufs=2)
                            for dchunk in range(N_CORES):
                                nc.tensor.matmul(
                                    ops[:, 0:gw], omT[:, dchunk, :],
                                    wo_sb[:, dchunk, csl],
                                    start=(dchunk == 0),
                                    stop=(dchunk == N_CORES - 1))
                            o_sb = pc2.tile([P, 512], bf16, tag="o_sb",
                                            bufs=2)
                            nc.scalar.copy(o_sb[:, 0:gw], ops[:, 0:gw])
                            nc.sync.dma_start(
                                out_d[half * P:(half + 1) * P, csl],
                                o_sb[:, 0:gw])

                    emit_wo(0, omTA)
                    emit_wo(1, omTB)

                if debug:
                    dqt = pp.tile([P, L], f32, tag="dqt")
                    nc.vector.tensor_copy(dqt[:], qTs[0][:])
                    nc.sync.dma_start(dbg["d_qT0"][:], dqt[:])
                    nc.vector.tensor_copy(dqt[:], kT[:])
                    nc.sync.dma_start(dbg["d_kT"][:], dqt[:])
                    dvm = pp.tile([P, D], f32, tag="dvm")
                    nc.vector.tensor_copy(dvm[:], vm[:, 3, :])
                    nc.sync.dma_start(dbg["d_vm"][:], dvm[:])
                    nc.sync.dma_start(dbg["d_r1"][:], rbuf1[:])
                    nc.sync.dma_start(dbg["d_r2"][:], rbuf2[:])
                    nc.sync.dma_start(dbg["d_ssq"][:], ssqb[:])
                    nc.sync.dma_start(dbg["d_scl"][:], scl[:])
                    nc.sync.dma_start(dbg["d_out1"][:], out1_all[:])
                    nc.vector.tensor_copy(dqt[:], onT[:])
                    nc.sync.dma_start(dbg["d_onT"][:], dqt[:])

    return nc


_CACHE = {}


def _get_program():
    if "nc" not in _CACHE:
        nc = _build()
        nc.compile()
        _CACHE["nc"] = nc
    return _CACHE["nc"]


def _host_lambda(x2, cos, sin, Wq, Wk, lq1, lk1, lq2, lk2):
    """Exact lambda: the dots are linear+rope functionals of x."""
    c32 = cos[:L, :32]
    s32 = sin[:L, :32]

    def rt(lam):
        l1, l2 = lam[:32], lam[32:]
        return np.concatenate([c32 * l1 + s32 * l2, -s32 * l1 + c32 * l2],
                              axis=1)  # [L, 64] rows R_l^T lam

    Wq3 = Wq.reshape(HID, H, D)
    Wqe = Wq3[:, 0::2, :].sum(axis=1)
    Wqo = Wq3[:, 1::2, :].sum(axis=1)
    Wks = Wk.reshape(HID, HKV, D).sum(axis=1)
    pk = x2 @ Wks
    d1 = np.clip(((x2 @ Wqe) * rt(lq1)).sum() / L, -10.0, 10.0)
    d2 = np.clip(2.0 * (pk * rt(lk1)).sum() / L, -10.0, 10.0)
    d3 = np.clip(((x2 @ Wqo) * rt(lq2)).sum() / L, -10.0, 10.0)
    d4 = np.clip(2.0 * (pk * rt(lk2)).sum() / L, -10.0, 10.0)
    lam = np.exp(d1) * np.exp(d2) - np.exp(d3) * np.exp(d4) + LAMBDA_INIT
    return float(np.clip(lam, 0.0, 1.0))


def _host_prep(x, cos, sin, Wq, Wk, Wv, Wo, lambda_q1, lambda_k1, lambda_q2,
               lambda_k2, subln_weight):
    bf = ml_dtypes.bfloat16
    x2 = np.asarray(x, np.float32).reshape(L, HID)
    cos = np.asarray(cos, np.float32)
    sin = np.asarray(sin, np.float32)
    # xt swizzled for 512B-contiguous lg-major streaming:
    # xtp[lg, ktp, p, e*128+col] = x2[lg*128+col, (2*ktp+e)*128+p]
    xT = np.ascontiguousarray(x2.T)                      # [HID, L]
    Bv = xT.reshape(8, 2, P, NQB, P)                     # [ktp, e, p, lg, col]
    xtp = np.ascontiguousarray(
        Bv.transpose(3, 0, 2, 1, 4).reshape(NQB * 8 * P, 256)).astype(bf)
    ropet = np.ascontiguousarray(
        np.concatenate([cos[:L, :32], sin[:L, :32]], axis=1))  # [L, 64]
    Wq = np.asarray(Wq, np.float32)
    Wk = np.asarray(Wk, np.float32)
    Wv = np.asarray(Wv, np.float32)
    lq1 = np.asarray(lambda_q1, np.float32)
    lq2 = np.asarray(lambda_q2, np.float32)
    lk1 = np.asarray(lambda_k1, np.float32)
    lk2 = np.asarray(lambda_k2, np.float32)
    lam = _host_lambda(x2, cos, sin, Wq, Wk, lq1, lk1, lq2, lk2)
    lamneg = np.full((1, 1), -lam, np.float32)
    s = np.asarray(subln_weight, np.float32) * (1.0 - LAMBDA_INIT)   # [128]
    Wo = np.asarray(Wo, np.float32)
    wo_eff = np.empty((H * D // 2, HID), np.float32)
    for p in range(H // 2):
        blk = Wo[p * 2 * D:(p + 1) * 2 * D, :]           # [128, HID]
        wo_eff[p * D:(p + 1) * D] = (s[:D, None] * blk[:D]
                                     + s[D:, None] * blk[D:])
    wo_eff = wo_eff.astype(bf)
    in_maps = []
    for c in range(N_CORES):
        wqkv_c = np.concatenate([
            Wq[:, c * NH * D:(c + 1) * NH * D],
            Wk[:, c * D:(c + 1) * D],
            Wv[:, c * D:(c + 1) * D]], axis=1).astype(bf)    # [HID, 384]
        in_maps.append({
            "xtp": xtp, "wqkv": wqkv_c, "ropet": ropet, "lamneg": lamneg,
            "wo": wo_eff,
        })
    return in_maps


def kernel(**inputs) -> np.ndarray:
    nc = _get_program()
    in_maps = _host_prep(**{k: v for k, v in inputs.items() if k != "mask"})
    res = run_bass_kernel_spmd(nc, in_maps, list(range(N_CORES)))
    out = np.empty((L, HID), np.float32)
    for c in range(N_CORES):
        r = np.asarray(res.results[c]["out"], np.float32)
        out[c * P:(c + 1) * P] = r[0:P]
        out[(c + 8) * P:(c + 9) * P] = r[P:2 * P]
    return out.reshape(1, L, HID)
